# revision 1
# baseline (speedup 1.0000x reference)
"""Trainium2 Bass kernel for 4-head spatial self-attention.

Computation (per batch b):
    xf = x[b] reshaped [C=256, n=4096]
    q/k/v = Wq/Wk/Wv @ xf            -> [128, n]   (rows = 4 heads x 32 dims)
    S_h   = (q_h^T k_h) * 32^-0.5    -> [n, n] per head
    P     = exp(S)  (softmax without max-subtraction: logits are O(5), safe in fp32)
    A_h   = v_h @ P_h^T / rowsum     -> [32, n]
    out   = Wout @ A + bout          -> [C, n]

Sharding: 8 cores = 4 batches x 2 query-halves. Each core handles all 4 heads
for one batch and 2048 queries vs all 4096 keys; outputs are disjoint slices.

Device layout (no transposes anywhere):
 - S is computed TRANSPOSED (keys on partitions, queries free) with the d=32
   head contractions packed onto PE row strips via tile_position (32h, 0).
   HW constraint (probed): concurrent row-strip matmuls MUST write different
   PSUM banks, so heads run in pairs and each head's S^T tile gets its own
   bank ([128, 2, 512] st tile).
 - exp(SCALE * S^T) runs on ScalarE PSUM->SBUF (the bottleneck: ~33.5M
   elements/core at 1 elem/lane/cycle, FD=1024 per op).
 - P@V accumulates over key chunks (K=128) on PE col strips; the PV weights
   (pre-transposed v from the projection x^T @ Wv^T) carry an extra ones
   column (M=33), so row 32 of each strip accumulates the softmax denominator
   for free. A K=1 matmul then broadcasts that row across partitions and a
   DVE divide normalizes.
"""

import numpy as np
import sys

for _p in ("/opt/trn_rl_repo", "/opt/pypackages"):
    if _p not in sys.path:
        sys.path.append(_p)

import concourse.bass as bass
import concourse.tile as tile
from concourse import bacc, mybir
from concourse.tile import add_dep_helper
from concourse.bass_utils import run_bass_kernel_spmd

f32 = mybir.dt.float32

B = 4
C = 256
N = 4096          # h*w = 64*64 key positions
NQ = 2048         # queries per core (half batch)
HEADS = 4
DH = 32
INNER = 128
SCALE = DH ** -0.5

QB = 512          # query block (free dim of S^T tiles)
NQB = NQ // QB    # 4
JT = 128          # key tile (partition dim of S^T tiles)
NJT = N // JT     # 32


def build_nc():
    nc = bacc.Bacc()

    xkv_d = nc.dram_tensor("xkv", [C, N], f32, kind="ExternalInput")
    xq_d = nc.dram_tensor("xq", [C, NQ], f32, kind="ExternalInput")
    wqT_d = nc.dram_tensor("wqT", [C, INNER], f32, kind="ExternalInput")
    wkT_d = nc.dram_tensor("wkT", [C, INNER], f32, kind="ExternalInput")
    wvT_d = nc.dram_tensor("wvT", [C, INNER], f32, kind="ExternalInput")
    # per-pair Wout^T with zero rows where `an` has no data:
    # rows 0:32 -> head 2p, rows 64:96 -> head 2p+1
    wo0_d = nc.dram_tensor("wo0", [INNER, C], f32, kind="ExternalInput")
    wo1_d = nc.dram_tensor("wo1", [INNER, C], f32, kind="ExternalInput")
    biasT_d = nc.dram_tensor("biasT", [128, 2], f32, kind="ExternalInput")
    out_d = nc.dram_tensor("out", [C, NQ], f32, kind="ExternalOutput")

    with tile.TileContext(nc) as tc:
        import contextlib

        ctx = contextlib.ExitStack()
        with ctx:
            big = ctx.enter_context(tc.tile_pool(name="big", bufs=1))
            wk = ctx.enter_context(tc.tile_pool(name="wk", bufs=2))
            ptp = ctx.enter_context(tc.tile_pool(name="ptp", bufs=3))
            ps_misc = ctx.enter_context(tc.tile_pool(name="ps_misc", bufs=2, space="PSUM"))
            ps_st = ctx.enter_context(tc.tile_pool(name="ps_st", bufs=2, space="PSUM"))
            ps_acc = ctx.enter_context(tc.tile_pool(name="ps_acc", bufs=2, space="PSUM"))

            # ---- constants / weights ----
            wqT_sb = big.tile([128, 2, INNER], f32)   # [c_part, c_chunk, inner]
            wkT_sb = big.tile([128, 2, INNER], f32)
            wvT_sb = big.tile([128, 2, INNER], f32)
            wo_sb = big.tile([128, 2, C], f32)        # [inner, pair, c]
            bias_sb = big.tile([128, 2], f32)
            ones1_sb = big.tile([128, 128], f32)      # row 0 used as [1,128] ones
            nc.vector.memset(ones1_sb[:], 1.0)
            for cc in range(2):
                nc.sync.dma_start(out=wqT_sb[:, cc, :], in_=wqT_d[128 * cc:128 * (cc + 1), :])
                nc.sync.dma_start(out=wkT_sb[:, cc, :], in_=wkT_d[128 * cc:128 * (cc + 1), :])
                nc.sync.dma_start(out=wvT_sb[:, cc, :], in_=wvT_d[128 * cc:128 * (cc + 1), :])
            nc.sync.dma_start(out=wo_sb[:, 0, :], in_=wo0_d[:])
            nc.sync.dma_start(out=wo_sb[:, 1, :], in_=wo1_d[:])
            nc.sync.dma_start(out=bias_sb[:], in_=biasT_d[:])

            # ---- activations in ----
            xkv_sb = big.tile([128, 2, N], f32)   # [c_part, c_chunk, n]
            xq_sb = big.tile([128, 2, NQ], f32)
            for cc in range(2):
                nc.sync.dma_start(out=xkv_sb[:, cc, :], in_=xkv_d[128 * cc:128 * (cc + 1), :])
                nc.sync.dma_start(out=xq_sb[:, cc, :], in_=xq_d[128 * cc:128 * (cc + 1), :])

            k_sb = big.tile([128, N], f32)     # [inner, n]
            q_sb = big.tile([128, NQ], f32)    # [inner, nq]
            # v^T chunks + ones col: [j0, (jtile, head), 33]; col 32 stays 1.0
            vT3 = big.tile([128, NJT * HEADS, DH + 1], f32)
            nc.vector.memset(vT3[:], 1.0)

            # ---- projections ----
            # k = Wk @ xkv ; q = Wq @ xq   (accumulate over the two C chunks)
            for t in range(N // 512):
                kp = ps_misc.tile([128, 512], f32, tag="misc", name="kp")
                for cc in range(2):
                    nc.tensor.matmul(
                        out=kp[:],
                        lhsT=wkT_sb[:, cc, :],
                        rhs=xkv_sb[:, cc, 512 * t:512 * (t + 1)],
                        start=(cc == 0), stop=(cc == 1),
                    )
                nc.scalar.copy(out=k_sb[:, 512 * t:512 * (t + 1)], in_=kp[:])
            for t in range(NQ // 512):
                qp = ps_misc.tile([128, 512], f32, tag="misc", name="qp")
                for cc in range(2):
                    nc.tensor.matmul(
                        out=qp[:],
                        lhsT=wqT_sb[:, cc, :],
                        rhs=xq_sb[:, cc, 512 * t:512 * (t + 1)],
                        start=(cc == 0), stop=(cc == 1),
                    )
                nc.vector.tensor_copy(out=q_sb[:, 512 * t:512 * (t + 1)], in_=qp[:])
            # vT[n, inner] = x^T @ Wv^T, 128-row tiles of n; 4 tiles per bank,
            # then one strided copy into the 33-col-stride augmented layout
            for T in range(N // 512):
                vp = ps_misc.tile([128, 4, 128], f32, tag="misc", name="vp")
                for t2 in range(4):
                    t = 4 * T + t2
                    for cc in range(2):
                        nc.tensor.matmul(
                            out=vp[:, t2, :],
                            lhsT=xkv_sb[:, cc, 128 * t:128 * (t + 1)],
                            rhs=wvT_sb[:, cc, :],
                            start=(cc == 0), stop=(cc == 1),
                        )
                src = vp.rearrange("p t (h d) -> p (t h) d", d=DH)
                nc.vector.tensor_copy(
                    out=vT3[:, 16 * T:16 * (T + 1), 0:DH], in_=src
                )

            # ---- attention ----
            for qb in range(NQB):
                q0 = QB * qb
                an_list = []
                for p in range(2):
                    # acc bank per (qb, pair): head hh -> A rows 64hh..64hh+32,
                    # denominator row 64hh+32 (ones column of the PV weights)
                    acc = ps_acc.tile([128, QB], f32, tag="acc", name="acc")
                    pv_prev = None
                    for J in range(NJT):
                        st = ps_st.tile([128, 2, QB], f32, tag="st", name="st")
                        for hh in range(2):
                            h = 2 * p + hh
                            nc.tensor.matmul(
                                out=st[:, hh, :],
                                lhsT=k_sb[32 * h:32 * (h + 1), JT * J:JT * (J + 1)],
                                rhs=q_sb[32 * h:32 * (h + 1), q0:q0 + QB],
                                start=True, stop=True,
                                tile_position=(32 * h, 0),
                            )
                        pt = ptp.tile([128, 2, QB], f32, tag="pt", name="pt")
                        nc.scalar.activation(
                            out=pt[:], in_=st[:],
                            func=mybir.ActivationFunctionType.Exp,
                            scale=SCALE,
                        )
                        for hh in range(2):
                            h = 2 * p + hh
                            r0 = 64 * hh
                            mm = nc.tensor.matmul(
                                out=acc[r0:r0 + 33, :],
                                lhsT=vT3[:, HEADS * J + h, :],
                                rhs=pt[:, hh, :],
                                start=(J == 0), stop=(J == NJT - 1),
                                tile_position=(0, r0),
                                skip_group_check=True,
                            )
                            if pv_prev is not None:
                                add_dep_helper(mm.ins, pv_prev.ins, sync=False, reason="pv order")
                            pv_prev = mm
                    # normalize: an rows 64hh..64hh+32 = A_hh / l_hh
                    an = wk.tile([128, QB], f32, tag="an", name="an")
                    nc.vector.memset(an[:], 0.0)
                    for hh in range(2):
                        r0 = 64 * hh
                        lrow = wk.tile([128, QB], f32, tag="lrow", name="lrow")
                        nc.vector.tensor_copy(
                            out=lrow[r0 + 32:r0 + 33, :], in_=acc[r0 + 32:r0 + 33, :]
                        )
                        # K=1 matmul broadcasts the denominator row across all
                        # 128 partitions (lhsT/rhs both live on partition r0+32)
                        lrep = ps_misc.tile([128, QB], f32, tag="misc", name="lrep")
                        nc.tensor.matmul(
                            out=lrep[:],
                            lhsT=ones1_sb[r0 + 32:r0 + 33, :],
                            rhs=lrow[r0 + 32:r0 + 33, :],
                            start=True, stop=True,
                            tile_position=(r0 + 32, 0),
                        )
                        rcp = wk.tile([128, QB], f32, tag="rcp", name="rcp")
                        nc.vector.reciprocal(out=rcp[r0:r0 + 32, :], in_=lrep[r0:r0 + 32, :])
                        nc.vector.tensor_mul(
                            out=an[r0:r0 + 32, :], in0=acc[r0:r0 + 32, :], in1=rcp[r0:r0 + 32, :]
                        )
                    an_list.append(an)
                # out projection + bias
                for cb in range(2):
                    op = ps_misc.tile([128, QB], f32, tag="misc", name="op")
                    for p in range(2):
                        nc.tensor.matmul(
                            out=op[:],
                            lhsT=wo_sb[:, p, 128 * cb:128 * (cb + 1)],
                            rhs=an_list[p][:],
                            start=(p == 0), stop=(p == 1),
                        )
                    ob = wk.tile([128, QB], f32, tag="ob", name="ob")
                    nc.vector.tensor_scalar_add(
                        out=ob[:], in0=op[:], scalar1=bias_sb[:, cb:cb + 1]
                    )
                    nc.sync.dma_start(
                        out=out_d[128 * cb:128 * (cb + 1), q0:q0 + QB], in_=ob[:]
                    )

    nc.compile()
    return nc


_NC_CACHE = []


def _get_nc():
    if not _NC_CACHE:
        _NC_CACHE.append(build_nc())
    return _NC_CACHE[0]


def _make_in_maps(x, Wq, Wk, Wv, Wout, bout):
    xf = np.ascontiguousarray(x.reshape(B, C, N), dtype=np.float32)
    wqT = np.ascontiguousarray(Wq.T, dtype=np.float32)
    wkT = np.ascontiguousarray(Wk.T, dtype=np.float32)
    wvT = np.ascontiguousarray(Wv.T, dtype=np.float32)
    woutT = np.asarray(Wout.T, dtype=np.float32)  # [inner, C]
    wo = []
    for p in range(2):
        m = np.zeros((INNER, C), dtype=np.float32)
        m[0:32] = woutT[64 * p:64 * p + 32]        # head 2p   -> an rows 0:32
        m[64:96] = woutT[64 * p + 32:64 * p + 64]  # head 2p+1 -> an rows 64:96
        wo.append(m)
    biasT = np.ascontiguousarray(bout.reshape(2, 128).T, dtype=np.float32)
    in_maps = []
    for core in range(8):
        b, half = core // 2, core % 2
        q0 = half * NQ
        in_maps.append({
            "xkv": xf[b],
            "xq": np.ascontiguousarray(xf[b][:, q0:q0 + NQ]),
            "wqT": wqT, "wkT": wkT, "wvT": wvT,
            "wo0": wo[0], "wo1": wo[1], "biasT": biasT,
        })
    return in_maps


def kernel(x, Wq, Wk, Wv, Wout, bout):
    nc = _get_nc()
    in_maps = _make_in_maps(x, Wq, Wk, Wv, Wout, bout)
    res = run_bass_kernel_spmd(nc, in_maps, core_ids=list(range(8)))
    out = np.empty((B, C, N), dtype=np.float32)
    for core in range(8):
        b, half = core // 2, core % 2
        q0 = half * NQ
        out[b][:, q0:q0 + NQ] = res.results[core]["out"]
    return out.reshape(B, C, 64, 64)



# revision 14
# speedup vs baseline: 3.2193x; 3.2193x over previous
"""Trainium2 Bass kernel for 4-head spatial self-attention.

Computation (per batch b):
    xf = x[b] reshaped [C=256, n=4096]
    q/k/v = Wq/Wk/Wv @ xf            -> [128, n]   (rows = 4 heads x 32 dims)
    S_h   = (q_h^T k_h) * 32^-0.5    -> [n, n] per head
    P     = exp(S)  (softmax without max-subtraction: logits are O(10), safe)
    A_h   = P_h V_h^T / rowsum       -> [n, 32]
    out   = Wout @ A + bout          -> [C, n]

Sharding: 8 cores = 4 batches x 2 query-halves. Each core handles all 4 heads
for one batch and 2048 queries vs all 4096 keys; outputs are disjoint slices.

Perf notes (cost-model driven):
 - All matmul operands are float32r or bf16: 1 PE cycle per output free-row
   (fp32 costs 4).  float32r needs moving-free >= 256, so the small-free
   matmuls (PV, transposes) use bf16; q/k stay f32r for exact logits.
 - S^T is computed with keys on partitions (queries free) so exp(S^T) tiles
   feed PV directly as stationary operands.  The d=32 head contractions pack
   onto PE row strips via tile_position; concurrent strip matmuls must write
   different PSUM banks ([128, 2, 512] st tile, one bank per head).
 - PV is computed TRANSPOSED: A^T[128q, 33] += pt_slice.T @ [V^T | 1].  The
   moving operand is 33 wide (vs 512 the other way round).  Column 32 of the
   rhs is ones, so A^T col 32 accumulates the softmax denominator for free.
 - exp is split across ScalarE (true exp, PSUM->SBUF bf16) and DVE
   (Schraudolph exp2: one tensor_scalar mult+add writing int16 bits that
   reinterpret as bf16 ~= 2^y).
 - The (pair, J) stream is software-pipelined: PV lags S/exp by PV_LAG steps
   so the in-order PE queue never waits on the exp engines.
 - A^T normalization: DVE copies acc->SBUF + reciprocal of the denominator
   column; the 16 per-(head,qs) scale-multiplies run on otherwise-idle
   GPSIMD (SBUF only).  A^T is transposed back to [hd, q] with full-128
   PE transposes (bf16) and projected with Wout^T stationary.
"""

import numpy as np
import sys

for _p in ("/opt/trn_rl_repo", "/opt/pypackages"):
    if _p not in sys.path:
        sys.path.append(_p)

import concourse.bass as bass
import concourse.tile as tile
from concourse import bacc, mybir
from concourse.tile import add_dep_helper
from concourse.bass_utils import run_bass_kernel_spmd

f32 = mybir.dt.float32
f32r = mybir.dt.float32r
bf16 = mybir.dt.bfloat16
i16 = mybir.dt.int16

B = 4
C = 256
N = 4096          # h*w = 64*64 key positions
NQ = 2048         # queries per core (half batch)
HEADS = 4
DH = 32
INNER = 128
SCALE = DH ** -0.5

QB = 512          # query block (free dim of S^T tiles)
NQB = NQ // QB    # 4
JT = 128          # key tile (partition dim of S^T tiles)
NJT = N // JT     # 32

# Schraudolph exp2 constants for the bf16 bit pattern:
#   bf16_bits(e^(S*SCALE)) ~= round(128*(S*SCALE*log2(e)) + 128*(127-sigma))
EXP2_SIGMA = 0.0435
EXP2_A = 128.0 * SCALE * 1.4426950408889634
EXP2_B = 128.0 * (127.0 - EXP2_SIGMA)

ACT_SHARE = 1.0  # DEBUG: all exp on ScalarE
PV_LAG = 8        # steps PV trails S/exp in the software pipeline


def build_nc():
    nc = bacc.Bacc()

    x_d = nc.dram_tensor("x", [C, N], bf16, kind="ExternalInput")
    xq_d = nc.dram_tensor("xq", [C, NQ], bf16, kind="ExternalInput")
    wqT_d = nc.dram_tensor("wqT", [C, INNER], bf16, kind="ExternalInput")
    wkT_d = nc.dram_tensor("wkT", [C, INNER], bf16, kind="ExternalInput")
    wvT_d = nc.dram_tensor("wvT", [C, INNER], bf16, kind="ExternalInput")
    woT_d = nc.dram_tensor("woT", [INNER, C], bf16, kind="ExternalInput")
    eye_d = nc.dram_tensor("eye", [128, 128], bf16, kind="ExternalInput")
    biasT_d = nc.dram_tensor("biasT", [128, 2], f32, kind="ExternalInput")
    out_d = nc.dram_tensor("out", [C, NQ], f32, kind="ExternalOutput")
    import os
    _dbg = os.environ.get("KDBG", "0") == "1"
    if _dbg:
        dbg_k = nc.dram_tensor("dbg_k", [128, N], f32, kind="ExternalOutput")
        dbg_q = nc.dram_tensor("dbg_q", [128, NQ], f32, kind="ExternalOutput")
        dbg_v3 = nc.dram_tensor("dbg_v3", [128, NJT * HEADS * (DH + 1)], f32, kind="ExternalOutput")
        dbg_pt = nc.dram_tensor("dbg_pt", [128, 2 * QB], f32, kind="ExternalOutput")
        dbg_atb = nc.dram_tensor("dbg_atb", [128, 16 * DH], f32, kind="ExternalOutput")
        dbg_acc = nc.dram_tensor("dbg_acc", [128, 8 * (DH + 1)], f32, kind="ExternalOutput")
        dbg_a = nc.dram_tensor("dbg_a", [128, 512], f32, kind="ExternalOutput")

    with tile.TileContext(nc) as tc:
        import contextlib

        ctx = contextlib.ExitStack()
        with ctx:
            big = ctx.enter_context(tc.tile_pool(name="big", bufs=1))
            wkp = ctx.enter_context(tc.tile_pool(name="wkp", bufs=2))
            ptp = ctx.enter_context(tc.tile_pool(name="ptp", bufs=PV_LAG + 2))
            ps = ctx.enter_context(tc.tile_pool(name="ps", bufs=2, space="PSUM"))

            # ---- constants / weights ----
            wq_sb = big.tile([128, 2, INNER], bf16)   # [c_part, c_chunk, inner]
            wk_sb = big.tile([128, 2, INNER], bf16)
            wv_sb = big.tile([128, 2, INNER], bf16)
            wo_sb = big.tile([128, C], bf16)          # [inner, c]
            eye_sb = big.tile([128, 128], bf16)
            bias_sb = big.tile([128, 2], f32)
            for cc in range(2):
                nc.sync.dma_start(out=wq_sb[:, cc, :], in_=wqT_d[128 * cc:128 * (cc + 1), :])
                nc.sync.dma_start(out=wk_sb[:, cc, :], in_=wkT_d[128 * cc:128 * (cc + 1), :])
                nc.sync.dma_start(out=wv_sb[:, cc, :], in_=wvT_d[128 * cc:128 * (cc + 1), :])
            nc.sync.dma_start(out=wo_sb[:], in_=woT_d[:])
            nc.sync.dma_start(out=eye_sb[:], in_=eye_d[:])
            nc.sync.dma_start(out=bias_sb[:], in_=biasT_d[:])

            # ---- activation DMA (chunked, interleaved with projections) ----
            x_sb = big.tile([128, 2, N], bf16)    # [c_part, c_chunk, n]
            xq_sb = big.tile([128, 2, NQ], bf16)

            k_sb = big.tile([128, N], f32r)       # [inner, n]
            q_sb = big.tile([128, NQ], f32r)      # [inner, nq]
            v_sb = big.tile([128, N], bf16)       # [inner, n]
            # v^T + ones col: [j0, (jtile, head), 33]; col 32 stays 1.0
            vT3 = big.tile([128, NJT * HEADS, DH + 1], bf16)
            nc.vector.memset(vT3[:, :, DH:DH + 1], 1.0)

            # ---- projections (bf16 inputs: 1 cycle/row), x DMA'd in 1024-col
            # chunks so the first matmuls start early ----
            for ch in range(4):
                c0 = 1024 * ch
                for cc in range(2):
                    nc.sync.dma_start(
                        out=x_sb[:, cc, c0:c0 + 1024],
                        in_=x_d[128 * cc:128 * (cc + 1), c0:c0 + 1024],
                    )
                for t2 in range(2):
                    t = 2 * ch + t2
                    kp = ps.tile([128, 2, 512], f32, tag="st", bufs=3, name="kp")[:, 0, :]
                    for cc in range(2):
                        nc.tensor.matmul(
                            out=kp[:],
                            lhsT=wk_sb[:, cc, :],
                            rhs=x_sb[:, cc, 512 * t:512 * (t + 1)],
                            start=(cc == 0), stop=(cc == 1),
                        )
                    nc.scalar.copy(out=k_sb[:, 512 * t:512 * (t + 1)], in_=kp[:])
                    vp = ps.tile([128, 2, 512], f32, tag="st", bufs=3, name="vp")[:, 0, :]
                    for cc in range(2):
                        nc.tensor.matmul(
                            out=vp[:],
                            lhsT=wv_sb[:, cc, :],
                            rhs=x_sb[:, cc, 512 * t:512 * (t + 1)],
                            start=(cc == 0), stop=(cc == 1),
                        )
                    nc.vector.tensor_copy(out=v_sb[:, 512 * t:512 * (t + 1)], in_=vp[:])
            for cc in range(2):
                nc.sync.dma_start(out=xq_sb[:, cc, :], in_=xq_d[128 * cc:128 * (cc + 1), :])
            for t in range(NQ // 512):
                qp = ps.tile([128, 2, 512], f32, tag="st", bufs=3, name="qp")[:, 0, :]
                for cc in range(2):
                    nc.tensor.matmul(
                        out=qp[:],
                        lhsT=wq_sb[:, cc, :],
                        rhs=xq_sb[:, cc, 512 * t:512 * (t + 1)],
                        start=(cc == 0), stop=(cc == 1),
                    )
                nc.scalar.copy(out=q_sb[:, 512 * t:512 * (t + 1)], in_=qp[:])

            # ---- v transpose: vT3[n, (J,h), 0:32] = v^T via full-128 PE
            # transposes (no row-strip hazards), 4 key-tiles per PSUM tile ----
            for T in range(N // 512):
                vp2_t = ps.tile([128, 2, 512], f32, tag="st", bufs=3, name="vp2")
                vp2 = vp2_t[:, 0, :].bitcast(bf16).rearrange("p (a b) -> p a b", b=128)
                for j2 in range(4):
                    j = 4 * T + j2
                    nc.tensor.transpose(
                        out=vp2[:, j2:j2 + 1, :],
                        in_=v_sb[:, 128 * j:128 * (j + 1)],
                        identity=eye_sb[:],
                    )
                src = vp2[:, 0:4, :].rearrange("p j (h d) -> p (j h) d", d=DH)
                nc.vector.tensor_copy(
                    out=vT3[:, 16 * T:16 * (T + 1), 0:DH], in_=src
                )

            if _dbg:
                dk = wkp.tile([128, N], f32, tag="dbgk", name="dbgk")
                nc.vector.tensor_copy(out=dk[:], in_=k_sb[:])
                nc.sync.dma_start(out=dbg_k[:], in_=dk[:])
                dq = wkp.tile([128, NQ], f32, tag="dbgq", name="dbgq")
                nc.vector.tensor_copy(out=dq[:], in_=q_sb[:])
                nc.sync.dma_start(out=dbg_q[:], in_=dq[:])
                dv3 = wkp.tile([128, NJT * HEADS * (DH + 1)], f32, tag="dbgv", name="dbgv")
                nc.vector.tensor_copy(out=dv3[:], in_=vT3.rearrange("p a b -> p (a b)"))
                nc.sync.dma_start(out=dbg_v3[:], in_=dv3[:])

            # ---- attention: software-pipelined (qb, p, J) stream ----
            steps = [(qb, p, J) for qb in range(NQB) for p in range(2)
                     for J in range(NJT)]
            nsteps = len(steps)
            exp_err = 0.0
            pt_tiles = {}     # step idx -> pt tile
            acc_tiles = {}    # (qb, p) -> acc psum tile
            atb_tiles = {}    # qb -> normalized A^T sbuf tile
            pv_prev = None
            # scheduled tail work: emitted after the S/PV of the given step
            tail_pair = {}    # step idx -> (qb, p) whose PVs just finished
            tail_qb = {}      # step idx -> qb ready for transpose/proj/out

            for i in range(nsteps + PV_LAG + 5):
                # S^T + exp for step i
                if i < nsteps:
                    qb, p, J = steps[i]
                    q0 = QB * qb
                    if (qb, p) not in acc_tiles:
                        acc_t = ps.tile([128, 512], f32, tag="acc", bufs=2, name="acc")
                        acc_tiles[(qb, p)] = acc_t[:, 0:8 * (DH + 1)].rearrange(
                            "p (a b) -> p a b", b=DH + 1
                        )
                    st = ps.tile([128, 2, QB], f32, tag="st", bufs=3, name="st")
                    for hh in range(2):
                        h = 2 * p + hh
                        nc.tensor.matmul(
                            out=st[:, hh, :],
                            lhsT=k_sb[32 * h:32 * (h + 1), JT * J:JT * (J + 1)],
                            rhs=q_sb[32 * h:32 * (h + 1), q0:q0 + QB],
                            start=True, stop=True,
                            tile_position=(32 * h, 0),
                        )
                    pt = ptp.tile([128, 2, QB], bf16, tag="pt", name="pt")
                    pt_tiles[i] = pt
                    dump_pt = _dbg and i == 0
                    exp_err += ACT_SHARE
                    if exp_err >= 1.0:
                        exp_err -= 1.0
                        nc.scalar.activation(
                            out=pt[:], in_=st[:],
                            func=mybir.ActivationFunctionType.Exp,
                            scale=SCALE,
                        )
                    else:
                        nc.vector.tensor_scalar(
                            out=pt[:].bitcast(i16), in0=st[:],
                            scalar1=EXP2_A, scalar2=EXP2_B,
                            op0=mybir.AluOpType.mult,
                            op1=mybir.AluOpType.add,
                        )
                    if dump_pt:
                        dpt = wkp.tile([128, 2 * QB], f32, tag="dbgpt", name="dbgpt")
                        nc.vector.tensor_copy(out=dpt[:], in_=pt.rearrange("p a b -> p (a b)"))
                        nc.sync.dma_start(out=dbg_pt[:], in_=dpt[:])

                # PV for step i - PV_LAG
                j = i - PV_LAG
                if 0 <= j < nsteps:
                    qb, p, J = steps[j]
                    acc = acc_tiles[(qb, p)]
                    pt = pt_tiles.pop(j)
                    for hh in range(2):
                        h = 2 * p + hh
                        for qs in range(4):
                            # start=True resets has_written for the WHOLE psum
                            # bank (values persist), so only the first matmul
                            # into the bank may set it; later groups' J=0
                            # writes land on cleared bits and start fresh.
                            mm = nc.tensor.matmul(
                                out=acc[:, 2 * qs + hh, :],
                                lhsT=pt[:, hh, 128 * qs:128 * (qs + 1)],
                                rhs=vT3[:, HEADS * J + h, :],
                                start=(J == 0 and hh == 0 and qs == 0),
                                stop=(J == NJT - 1),
                                skip_group_check=True,
                            )
                            if pv_prev is not None:
                                add_dep_helper(mm.ins, pv_prev.ins, sync=False,
                                               reason="pv order")
                            pv_prev = mm
                    if J == NJT - 1:
                        tail_pair[i + 1] = (qb, p)
                        if p == 1:
                            tail_qb[i + 4] = qb

                # per-pair normalization as soon as a pair's PVs are done:
                # DVE copies acc->SBUF + reciprocal; GPSIMD scales (SBUF only)
                if i in tail_pair:
                    qb, p = tail_pair.pop(i)
                    acc = acc_tiles.pop((qb, p))
                    acc_sb = wkp.tile([128, 8, DH + 1], f32, tag="accsb", name="acc_sb")
                    nc.vector.tensor_copy(out=acc_sb[:], in_=acc[:])
                    if _dbg and qb == 0 and p == 0:
                        nc.sync.dma_start(out=dbg_acc[:], in_=acc_sb.rearrange("p a b -> p (a b)"))
                    rcp_sb = wkp.tile([128, 8], f32, tag="rcp", name="rcp")
                    nc.vector.reciprocal(out=rcp_sb[:], in_=acc_sb[:, :, DH])
                    if qb not in atb_tiles:
                        atb_tiles[qb] = wkp.tile([128, 16, DH], bf16, tag="atb", name="atb")
                    atb = atb_tiles[qb]
                    for qs in range(4):
                        for hh in range(2):
                            nc.gpsimd.tensor_scalar_mul(
                                out=atb[:, 4 * qs + 2 * p + hh, :],
                                in0=acc_sb[:, 2 * qs + hh, 0:DH],
                                scalar1=rcp_sb[:, 2 * qs + hh:2 * qs + hh + 1],
                            )

                # per-qb finish: transpose A^T -> A, project, bias, DMA out
                if i in tail_qb:
                    qb = tail_qb.pop(i)
                    q0 = QB * qb
                    atb = atb_tiles.pop(qb)
                    a_ps_t = ps.tile([128, 2, 512], f32, tag="st", bufs=3, name="a_ps")
                    a_ps = a_ps_t[:, 0, :].bitcast(bf16).rearrange("p (a b) -> p a b", b=128)
                    atb_flat = atb.rearrange("p i d -> p (i d)")
                    for qs in range(4):
                        nc.tensor.transpose(
                            out=a_ps[:, qs:qs + 1, :],
                            in_=atb_flat[:, 128 * qs:128 * (qs + 1)],
                            identity=eye_sb[:],
                        )
                    a_sb = wkp.tile([128, 4, 128], bf16, tag="asb", name="a_sb")
                    nc.vector.tensor_copy(out=a_sb[:], in_=a_ps[:, 0:4, :])
                    if _dbg and qb == 0:
                        datb = wkp.tile([128, 16 * DH], f32, tag="dbgatb", name="dbgatb")
                        nc.vector.tensor_copy(out=datb[:], in_=atb.rearrange("p a b -> p (a b)"))
                        nc.sync.dma_start(out=dbg_atb[:], in_=datb[:])
                        da = wkp.tile([128, 512], f32, tag="dbga", name="dbga")
                        nc.vector.tensor_copy(out=da[:], in_=a_sb.rearrange("p a b -> p (a b)"))
                        nc.sync.dma_start(out=dbg_a[:], in_=da[:])
                    a_flat = a_sb.rearrange("p a b -> p (a b)")
                    for cb in range(2):
                        op = ps.tile([128, 2, QB], f32, tag="st", bufs=3, name="op")[:, 0, :]
                        nc.tensor.matmul(
                            out=op[:],
                            lhsT=wo_sb[:, 128 * cb:128 * (cb + 1)],
                            rhs=a_flat[:],
                            start=True, stop=True,
                        )
                        ob = wkp.tile([128, QB], f32, tag="ob", name="ob")
                        nc.scalar.add(
                            out=ob[:], in_=op[:], add=bias_sb[:, cb:cb + 1]
                        )
                        nc.sync.dma_start(
                            out=out_d[128 * cb:128 * (cb + 1), q0:q0 + QB], in_=ob[:]
                        )

            # flush any remaining tails
            for i in sorted(list(tail_pair) + list(tail_qb)):
                assert False, "tails must be drained inside the loop"

    nc.compile()
    return nc


_NC_CACHE = []


def _get_nc():
    if not _NC_CACHE:
        _NC_CACHE.append(build_nc())
    return _NC_CACHE[0]


def _make_in_maps(x, Wq, Wk, Wv, Wout, bout):
    import ml_dtypes

    bfl = ml_dtypes.bfloat16
    xf = np.ascontiguousarray(x.reshape(B, C, N)).astype(bfl)
    wqT = np.ascontiguousarray(np.asarray(Wq, dtype=np.float32).T.astype(bfl))
    wkT = np.ascontiguousarray(np.asarray(Wk, dtype=np.float32).T.astype(bfl))
    wvT = np.ascontiguousarray(np.asarray(Wv, dtype=np.float32).T.astype(bfl))
    woT = np.ascontiguousarray(np.asarray(Wout, dtype=np.float32).T.astype(bfl))
    eye = np.eye(128, dtype=bfl)
    biasT = np.ascontiguousarray(
        np.asarray(bout, dtype=np.float32).reshape(2, 128).T
    )
    in_maps = []
    for core in range(8):
        b, half = core // 2, core % 2
        q0 = half * NQ
        in_maps.append({
            "x": xf[b],
            "xq": np.ascontiguousarray(xf[b][:, q0:q0 + NQ]),
            "wqT": wqT, "wkT": wkT, "wvT": wvT,
            "woT": woT, "eye": eye, "biasT": biasT,
        })
    return in_maps


def kernel(x, Wq, Wk, Wv, Wout, bout):
    nc = _get_nc()
    in_maps = _make_in_maps(x, Wq, Wk, Wv, Wout, bout)
    res = run_bass_kernel_spmd(nc, in_maps, core_ids=list(range(8)))
    out = np.empty((B, C, N), dtype=np.float32)
    for core in range(8):
        b, half = core // 2, core % 2
        q0 = half * NQ
        out[b][:, q0:q0 + NQ] = res.results[core]["out"]
    return out.reshape(B, C, 64, 64)


# revision 15
# speedup vs baseline: 4.6896x; 1.4567x over previous
"""Trainium2 Bass kernel for 4-head spatial self-attention.

Computation (per batch b):
    xf = x[b] reshaped [C=256, n=4096]
    q/k/v = Wq/Wk/Wv @ xf            -> [128, n]   (rows = 4 heads x 32 dims)
    S_h   = (q_h^T k_h) * 32^-0.5    -> [n, n] per head
    P     = exp(S)  (softmax without max-subtraction: logits are O(10), safe)
    A_h   = P_h V_h^T / rowsum       -> [n, 32]
    out   = Wout @ A + bout          -> [C, n]

Sharding: 8 cores = 4 batches x 2 query-halves. Each core handles all 4 heads
for one batch and 2048 queries vs all 4096 keys; outputs are disjoint slices.

Perf notes (cost-model driven):
 - All matmul operands are float32r or bf16: 1 PE cycle per output free-row
   (fp32 costs 4).  float32r needs moving-free >= 256, so the small-free
   matmuls (PV, transposes) use bf16; q/k stay f32r for exact logits.
 - S^T is computed with keys on partitions (queries free) so exp(S^T) tiles
   feed PV directly as stationary operands.  The d=32 head contractions pack
   onto PE row strips via tile_position; concurrent strip matmuls must write
   different PSUM banks ([128, 2, 512] st tile, one bank per head).
 - PV is computed TRANSPOSED: A^T[128q, 33] += pt_slice.T @ [V^T | 1].  The
   moving operand is 33 wide (vs 512 the other way round).  Column 32 of the
   rhs is ones, so A^T col 32 accumulates the softmax denominator for free.
 - exp is split across ScalarE (true exp, PSUM->SBUF bf16) and DVE
   (Schraudolph exp2: one tensor_scalar mult+add writing int16 bits that
   reinterpret as bf16 ~= 2^y).
 - The (pair, J) stream is software-pipelined: PV lags S/exp by PV_LAG steps
   so the in-order PE queue never waits on the exp engines.
 - A^T normalization: DVE copies acc->SBUF + reciprocal of the denominator
   column; the 16 per-(head,qs) scale-multiplies run on otherwise-idle
   GPSIMD (SBUF only).  A^T is transposed back to [hd, q] with full-128
   PE transposes (bf16) and projected with Wout^T stationary.
"""

import numpy as np
import sys

for _p in ("/opt/trn_rl_repo", "/opt/pypackages"):
    if _p not in sys.path:
        sys.path.append(_p)

import concourse.bass as bass
import concourse.tile as tile
from concourse import bacc, mybir
from concourse.tile import add_dep_helper
from concourse.bass_utils import run_bass_kernel_spmd

f32 = mybir.dt.float32
f32r = mybir.dt.float32r
bf16 = mybir.dt.bfloat16
i16 = mybir.dt.int16

B = 4
C = 256
N = 4096          # h*w = 64*64 key positions
NQ = 2048         # queries per core (half batch)
HEADS = 4
DH = 32
INNER = 128
SCALE = DH ** -0.5

QB = 512          # query block (free dim of S^T tiles)
NQB = NQ // QB    # 4
JT = 128          # key tile (partition dim of S^T tiles)
NJT = N // JT     # 32

# Schraudolph exp2 constants for the bf16 bit pattern:
#   bf16_bits(e^(S*SCALE)) ~= round(128*(S*SCALE*log2(e)) + 128*(127-sigma))
EXP2_SIGMA = 0.0435
EXP2_A = 128.0 * SCALE * 1.4426950408889634
EXP2_B = 128.0 * (127.0 - EXP2_SIGMA)

ACT_SHARE = 0.54  # fraction of exp tiles on ScalarE (rest on DVE)
PV_LAG = 8        # steps PV trails S/exp in the software pipeline


def build_nc():
    nc = bacc.Bacc()

    x_d = nc.dram_tensor("x", [C, N], bf16, kind="ExternalInput")
    xq_d = nc.dram_tensor("xq", [C, NQ], bf16, kind="ExternalInput")
    wqT_d = nc.dram_tensor("wqT", [C, INNER], bf16, kind="ExternalInput")
    wkT_d = nc.dram_tensor("wkT", [C, INNER], bf16, kind="ExternalInput")
    wvT_d = nc.dram_tensor("wvT", [C, INNER], bf16, kind="ExternalInput")
    woT_d = nc.dram_tensor("woT", [INNER, C], bf16, kind="ExternalInput")
    eye_d = nc.dram_tensor("eye", [128, 128], bf16, kind="ExternalInput")
    biasT_d = nc.dram_tensor("biasT", [128, 2], f32, kind="ExternalInput")
    out_d = nc.dram_tensor("out", [C, NQ], f32, kind="ExternalOutput")
    import os
    _dbg = os.environ.get("KDBG", "0") == "1"
    if _dbg:
        dbg_k = nc.dram_tensor("dbg_k", [128, N], f32, kind="ExternalOutput")
        dbg_q = nc.dram_tensor("dbg_q", [128, NQ], f32, kind="ExternalOutput")
        dbg_v3 = nc.dram_tensor("dbg_v3", [128, NJT * HEADS * (DH + 1)], f32, kind="ExternalOutput")
        dbg_pt = nc.dram_tensor("dbg_pt", [128, 2 * QB], f32, kind="ExternalOutput")
        dbg_atb = nc.dram_tensor("dbg_atb", [128, 16 * DH], f32, kind="ExternalOutput")
        dbg_acc = nc.dram_tensor("dbg_acc", [128, 8 * (DH + 1)], f32, kind="ExternalOutput")
        dbg_a = nc.dram_tensor("dbg_a", [128, 512], f32, kind="ExternalOutput")

    with tile.TileContext(nc) as tc:
        import contextlib

        ctx = contextlib.ExitStack()
        with ctx:
            big = ctx.enter_context(tc.tile_pool(name="big", bufs=1))
            wkp = ctx.enter_context(tc.tile_pool(name="wkp", bufs=2))
            ptp = ctx.enter_context(tc.tile_pool(name="ptp", bufs=PV_LAG + 2))
            ps = ctx.enter_context(tc.tile_pool(name="ps", bufs=2, space="PSUM"))

            # ---- constants / weights ----
            wq_sb = big.tile([128, 2, INNER], bf16)   # [c_part, c_chunk, inner]
            wk_sb = big.tile([128, 2, INNER], bf16)
            wv_sb = big.tile([128, 2, INNER], bf16)
            wo_sb = big.tile([128, C], bf16)          # [inner, c]
            eye_sb = big.tile([128, 128], bf16)
            bias_sb = big.tile([128, 2], f32)
            for cc in range(2):
                nc.sync.dma_start(out=wq_sb[:, cc, :], in_=wqT_d[128 * cc:128 * (cc + 1), :])
                nc.sync.dma_start(out=wk_sb[:, cc, :], in_=wkT_d[128 * cc:128 * (cc + 1), :])
                nc.sync.dma_start(out=wv_sb[:, cc, :], in_=wvT_d[128 * cc:128 * (cc + 1), :])
            nc.sync.dma_start(out=wo_sb[:], in_=woT_d[:])
            nc.sync.dma_start(out=eye_sb[:], in_=eye_d[:])
            nc.sync.dma_start(out=bias_sb[:], in_=biasT_d[:])

            # ---- activation DMA (chunked, interleaved with projections) ----
            x_sb = big.tile([128, 2, N], bf16)    # [c_part, c_chunk, n]
            xq_sb = big.tile([128, 2, NQ], bf16)

            k_sb = big.tile([128, N], f32r)       # [inner, n]
            q_sb = big.tile([128, NQ], f32r)      # [inner, nq]
            v_sb = big.tile([128, N], bf16)       # [inner, n]
            # v^T + ones col: [j0, (jtile, head), 33]; col 32 stays 1.0
            vT3 = big.tile([128, NJT * HEADS, DH + 1], bf16)
            nc.vector.memset(vT3[:, :, DH:DH + 1], 1.0)

            # ---- projections (bf16 inputs: 1 cycle/row), x DMA'd in 1024-col
            # chunks so the first matmuls start early ----
            for ch in range(4):
                c0 = 1024 * ch
                for cc in range(2):
                    nc.sync.dma_start(
                        out=x_sb[:, cc, c0:c0 + 1024],
                        in_=x_d[128 * cc:128 * (cc + 1), c0:c0 + 1024],
                    )
                for t2 in range(2):
                    t = 2 * ch + t2
                    kp = ps.tile([128, 2, 512], f32, tag="st", bufs=3, name="kp")[:, 0, :]
                    for cc in range(2):
                        nc.tensor.matmul(
                            out=kp[:],
                            lhsT=wk_sb[:, cc, :],
                            rhs=x_sb[:, cc, 512 * t:512 * (t + 1)],
                            start=(cc == 0), stop=(cc == 1),
                        )
                    nc.scalar.copy(out=k_sb[:, 512 * t:512 * (t + 1)], in_=kp[:])
                    vp = ps.tile([128, 2, 512], f32, tag="st", bufs=3, name="vp")[:, 0, :]
                    for cc in range(2):
                        nc.tensor.matmul(
                            out=vp[:],
                            lhsT=wv_sb[:, cc, :],
                            rhs=x_sb[:, cc, 512 * t:512 * (t + 1)],
                            start=(cc == 0), stop=(cc == 1),
                        )
                    nc.vector.tensor_copy(out=v_sb[:, 512 * t:512 * (t + 1)], in_=vp[:])
            for cc in range(2):
                nc.sync.dma_start(out=xq_sb[:, cc, :], in_=xq_d[128 * cc:128 * (cc + 1), :])
            for t in range(NQ // 512):
                qp = ps.tile([128, 2, 512], f32, tag="st", bufs=3, name="qp")[:, 0, :]
                for cc in range(2):
                    nc.tensor.matmul(
                        out=qp[:],
                        lhsT=wq_sb[:, cc, :],
                        rhs=xq_sb[:, cc, 512 * t:512 * (t + 1)],
                        start=(cc == 0), stop=(cc == 1),
                    )
                nc.scalar.copy(out=q_sb[:, 512 * t:512 * (t + 1)], in_=qp[:])

            # ---- v transpose: vT3[n, (J,h), 0:32] = v^T via full-128 PE
            # transposes (no row-strip hazards), 4 key-tiles per PSUM tile ----
            for T in range(N // 512):
                vp2_t = ps.tile([128, 2, 512], f32, tag="st", bufs=3, name="vp2")
                vp2 = vp2_t[:, 0, :].bitcast(bf16).rearrange("p (a b) -> p a b", b=128)
                for j2 in range(4):
                    j = 4 * T + j2
                    nc.tensor.transpose(
                        out=vp2[:, j2:j2 + 1, :],
                        in_=v_sb[:, 128 * j:128 * (j + 1)],
                        identity=eye_sb[:],
                    )
                src = vp2[:, 0:4, :].rearrange("p j (h d) -> p (j h) d", d=DH)
                nc.vector.tensor_copy(
                    out=vT3[:, 16 * T:16 * (T + 1), 0:DH], in_=src
                )

            if _dbg:
                dk = wkp.tile([128, N], f32, tag="dbgk", name="dbgk")
                nc.vector.tensor_copy(out=dk[:], in_=k_sb[:])
                nc.sync.dma_start(out=dbg_k[:], in_=dk[:])
                dq = wkp.tile([128, NQ], f32, tag="dbgq", name="dbgq")
                nc.vector.tensor_copy(out=dq[:], in_=q_sb[:])
                nc.sync.dma_start(out=dbg_q[:], in_=dq[:])
                dv3 = wkp.tile([128, NJT * HEADS * (DH + 1)], f32, tag="dbgv", name="dbgv")
                nc.vector.tensor_copy(out=dv3[:], in_=vT3.rearrange("p a b -> p (a b)"))
                nc.sync.dma_start(out=dbg_v3[:], in_=dv3[:])

            # ---- attention: software-pipelined (qb, p, J) stream ----
            steps = [(qb, p, J) for qb in range(NQB) for p in range(2)
                     for J in range(NJT)]
            nsteps = len(steps)
            exp_err = 0.0
            pt_tiles = {}     # step idx -> pt tile
            acc_tiles = {}    # (qb, p) -> acc psum tile
            atb_tiles = {}    # qb -> normalized A^T sbuf tile
            pv_prev = None
            # scheduled tail work: emitted after the S/PV of the given step
            tail_pair = {}    # step idx -> (qb, p) whose PVs just finished
            tail_qb = {}      # step idx -> qb ready for transpose/proj/out

            for i in range(nsteps + PV_LAG + 5):
                # S^T + exp for step i
                if i < nsteps:
                    qb, p, J = steps[i]
                    q0 = QB * qb
                    if (qb, p) not in acc_tiles:
                        acc_t = ps.tile([128, 512], f32, tag="acc", bufs=2, name="acc")
                        acc_tiles[(qb, p)] = acc_t[:, 0:8 * (DH + 1)].rearrange(
                            "p (a b) -> p a b", b=DH + 1
                        )
                    st = ps.tile([128, 2, QB], f32, tag="st", bufs=3, name="st")
                    for hh in range(2):
                        h = 2 * p + hh
                        nc.tensor.matmul(
                            out=st[:, hh, :],
                            lhsT=k_sb[32 * h:32 * (h + 1), JT * J:JT * (J + 1)],
                            rhs=q_sb[32 * h:32 * (h + 1), q0:q0 + QB],
                            start=True, stop=True,
                            tile_position=(32 * h, 0),
                        )
                    pt = ptp.tile([128, 2, QB], bf16, tag="pt", name="pt")
                    pt_tiles[i] = pt
                    dump_pt = _dbg and i == 0
                    exp_err += ACT_SHARE
                    if exp_err >= 1.0:
                        exp_err -= 1.0
                        nc.scalar.activation(
                            out=pt[:], in_=st[:],
                            func=mybir.ActivationFunctionType.Exp,
                            scale=SCALE,
                        )
                    else:
                        nc.vector.tensor_scalar(
                            out=pt[:].bitcast(i16), in0=st[:],
                            scalar1=EXP2_A, scalar2=EXP2_B,
                            op0=mybir.AluOpType.mult,
                            op1=mybir.AluOpType.add,
                        )
                    if dump_pt:
                        dpt = wkp.tile([128, 2 * QB], f32, tag="dbgpt", name="dbgpt")
                        nc.vector.tensor_copy(out=dpt[:], in_=pt.rearrange("p a b -> p (a b)"))
                        nc.sync.dma_start(out=dbg_pt[:], in_=dpt[:])

                # PV for step i - PV_LAG
                j = i - PV_LAG
                if 0 <= j < nsteps:
                    qb, p, J = steps[j]
                    acc = acc_tiles[(qb, p)]
                    pt = pt_tiles.pop(j)
                    for hh in range(2):
                        h = 2 * p + hh
                        for qs in range(4):
                            # start=True resets has_written for the WHOLE psum
                            # bank (values persist), so only the first matmul
                            # into the bank may set it; later groups' J=0
                            # writes land on cleared bits and start fresh.
                            mm = nc.tensor.matmul(
                                out=acc[:, 2 * qs + hh, :],
                                lhsT=pt[:, hh, 128 * qs:128 * (qs + 1)],
                                rhs=vT3[:, HEADS * J + h, :],
                                start=(J == 0 and hh == 0 and qs == 0),
                                stop=(J == NJT - 1),
                                skip_group_check=True,
                            )
                            if pv_prev is not None:
                                add_dep_helper(mm.ins, pv_prev.ins, sync=False,
                                               reason="pv order")
                            pv_prev = mm
                    if J == NJT - 1:
                        tail_pair[i + 1] = (qb, p)
                        if p == 1:
                            tail_qb[i + 4] = qb

                # per-pair normalization as soon as a pair's PVs are done:
                # DVE copies acc->SBUF + reciprocal; GPSIMD scales (SBUF only)
                if i in tail_pair:
                    qb, p = tail_pair.pop(i)
                    acc = acc_tiles.pop((qb, p))
                    acc_sb = wkp.tile([128, 8, DH + 1], f32, tag="accsb", name="acc_sb")
                    nc.vector.tensor_copy(out=acc_sb[:], in_=acc[:])
                    if _dbg and qb == 0 and p == 0:
                        nc.sync.dma_start(out=dbg_acc[:], in_=acc_sb.rearrange("p a b -> p (a b)"))
                    rcp_sb = wkp.tile([128, 8], f32, tag="rcp", name="rcp")
                    nc.vector.reciprocal(out=rcp_sb[:], in_=acc_sb[:, :, DH])
                    if qb not in atb_tiles:
                        atb_tiles[qb] = wkp.tile([128, 16, DH], bf16, tag="atb", name="atb")
                    atb = atb_tiles[qb]
                    for qs in range(4):
                        for hh in range(2):
                            nc.gpsimd.tensor_scalar_mul(
                                out=atb[:, 4 * qs + 2 * p + hh, :],
                                in0=acc_sb[:, 2 * qs + hh, 0:DH],
                                scalar1=rcp_sb[:, 2 * qs + hh:2 * qs + hh + 1],
                            )

                # per-qb finish: transpose A^T -> A, project, bias, DMA out
                if i in tail_qb:
                    qb = tail_qb.pop(i)
                    q0 = QB * qb
                    atb = atb_tiles.pop(qb)
                    a_ps_t = ps.tile([128, 2, 512], f32, tag="st", bufs=3, name="a_ps")
                    a_ps = a_ps_t[:, 0, :].bitcast(bf16).rearrange("p (a b) -> p a b", b=128)
                    atb_flat = atb.rearrange("p i d -> p (i d)")
                    for qs in range(4):
                        nc.tensor.transpose(
                            out=a_ps[:, qs:qs + 1, :],
                            in_=atb_flat[:, 128 * qs:128 * (qs + 1)],
                            identity=eye_sb[:],
                        )
                    a_sb = wkp.tile([128, 4, 128], bf16, tag="asb", name="a_sb")
                    nc.vector.tensor_copy(out=a_sb[:], in_=a_ps[:, 0:4, :])
                    if _dbg and qb == 0:
                        datb = wkp.tile([128, 16 * DH], f32, tag="dbgatb", name="dbgatb")
                        nc.vector.tensor_copy(out=datb[:], in_=atb.rearrange("p a b -> p (a b)"))
                        nc.sync.dma_start(out=dbg_atb[:], in_=datb[:])
                        da = wkp.tile([128, 512], f32, tag="dbga", name="dbga")
                        nc.vector.tensor_copy(out=da[:], in_=a_sb.rearrange("p a b -> p (a b)"))
                        nc.sync.dma_start(out=dbg_a[:], in_=da[:])
                    a_flat = a_sb.rearrange("p a b -> p (a b)")
                    for cb in range(2):
                        op = ps.tile([128, 2, QB], f32, tag="st", bufs=3, name="op")[:, 0, :]
                        nc.tensor.matmul(
                            out=op[:],
                            lhsT=wo_sb[:, 128 * cb:128 * (cb + 1)],
                            rhs=a_flat[:],
                            start=True, stop=True,
                        )
                        ob = wkp.tile([128, QB], f32, tag="ob", name="ob")
                        nc.scalar.add(
                            out=ob[:], in_=op[:], add=bias_sb[:, cb:cb + 1]
                        )
                        nc.sync.dma_start(
                            out=out_d[128 * cb:128 * (cb + 1), q0:q0 + QB], in_=ob[:]
                        )

            # flush any remaining tails
            for i in sorted(list(tail_pair) + list(tail_qb)):
                assert False, "tails must be drained inside the loop"

    nc.compile()
    return nc


_NC_CACHE = []


def _get_nc():
    if not _NC_CACHE:
        _NC_CACHE.append(build_nc())
    return _NC_CACHE[0]


def _make_in_maps(x, Wq, Wk, Wv, Wout, bout):
    import ml_dtypes

    bfl = ml_dtypes.bfloat16
    xf = np.ascontiguousarray(x.reshape(B, C, N)).astype(bfl)
    wqT = np.ascontiguousarray(np.asarray(Wq, dtype=np.float32).T.astype(bfl))
    wkT = np.ascontiguousarray(np.asarray(Wk, dtype=np.float32).T.astype(bfl))
    wvT = np.ascontiguousarray(np.asarray(Wv, dtype=np.float32).T.astype(bfl))
    woT = np.ascontiguousarray(np.asarray(Wout, dtype=np.float32).T.astype(bfl))
    eye = np.eye(128, dtype=bfl)
    biasT = np.ascontiguousarray(
        np.asarray(bout, dtype=np.float32).reshape(2, 128).T
    )
    in_maps = []
    for core in range(8):
        b, half = core // 2, core % 2
        q0 = half * NQ
        in_maps.append({
            "x": xf[b],
            "xq": np.ascontiguousarray(xf[b][:, q0:q0 + NQ]),
            "wqT": wqT, "wkT": wkT, "wvT": wvT,
            "woT": woT, "eye": eye, "biasT": biasT,
        })
    return in_maps


def kernel(x, Wq, Wk, Wv, Wout, bout):
    nc = _get_nc()
    in_maps = _make_in_maps(x, Wq, Wk, Wv, Wout, bout)
    res = run_bass_kernel_spmd(nc, in_maps, core_ids=list(range(8)))
    out = np.empty((B, C, N), dtype=np.float32)
    for core in range(8):
        b, half = core // 2, core % 2
        q0 = half * NQ
        out[b][:, q0:q0 + NQ] = res.results[core]["out"]
    return out.reshape(B, C, 64, 64)


# revision 24
# speedup vs baseline: 4.7840x; 1.0201x over previous
"""Trainium2 Bass kernel for 4-head spatial self-attention.

Computation (per batch b):
    xf = x[b] reshaped [C=256, n=4096]
    q/k/v = Wq/Wk/Wv @ xf            -> [128, n]   (rows = 4 heads x 32 dims)
    S_h   = (q_h^T k_h) * 32^-0.5    -> [n, n] per head
    P     = exp(S)  (softmax without max-subtraction: logits are O(10), safe)
    A_h   = P_h V_h^T / rowsum       -> [n, 32]
    out   = Wout @ A + bout          -> [C, n]

Sharding: 8 cores = 4 batches x 2 query-halves. Each core handles all 4 heads
for one batch and 2048 queries vs all 4096 keys; outputs are disjoint slices.

Perf notes (cost-model driven):
 - All matmul operands are float32r or bf16: 1 PE cycle per output free-row
   (fp32 costs 4).  float32r needs moving-free >= 256, so the small-free
   matmuls (PV, transposes) use bf16; q/k stay f32r for exact logits.
 - S^T is computed with keys on partitions (queries free) so exp(S^T) tiles
   feed PV directly as stationary operands.  The d=32 head contractions pack
   onto PE row strips via tile_position; concurrent strip matmuls must write
   different PSUM banks ([128, 2, 512] st tile, one bank per head).
 - PV is computed TRANSPOSED: A^T[128q, 33] += pt_slice.T @ [V^T | 1].  The
   moving operand is 33 wide (vs 512 the other way round).  Column 32 of the
   rhs is ones, so A^T col 32 accumulates the softmax denominator for free.
 - exp is split across ScalarE (true exp, PSUM->SBUF bf16) and DVE
   (Schraudolph exp2: one tensor_scalar mult+add writing int16 bits that
   reinterpret as bf16 ~= 2^y).
 - The (pair, J) stream is software-pipelined: PV lags S/exp by PV_LAG steps
   so the in-order PE queue never waits on the exp engines.
 - A^T normalization: DVE copies acc->SBUF + reciprocal of the denominator
   column; the 16 per-(head,qs) scale-multiplies run on otherwise-idle
   GPSIMD (SBUF only).  A^T is transposed back to [hd, q] with full-128
   PE transposes (bf16) and projected with Wout^T stationary.
"""

import numpy as np
import sys

for _p in ("/opt/trn_rl_repo", "/opt/pypackages"):
    if _p not in sys.path:
        sys.path.append(_p)

import concourse.bass as bass
import concourse.tile as tile
from concourse import bacc, mybir
from concourse.tile import add_dep_helper
from concourse.bass_utils import run_bass_kernel_spmd

f32 = mybir.dt.float32
f32r = mybir.dt.float32r
bf16 = mybir.dt.bfloat16
i16 = mybir.dt.int16

B = 4
C = 256
N = 4096          # h*w = 64*64 key positions
NQ = 2048         # queries per core (half batch)
HEADS = 4
DH = 32
INNER = 128
SCALE = DH ** -0.5

QB = 512          # query block (free dim of S^T tiles)
NQB = NQ // QB    # 4
JT = 128          # key tile (partition dim of S^T tiles)
NJT = N // JT     # 32

# Schraudolph exp2 constants for the bf16 bit pattern:
#   bf16_bits(e^(S*SCALE)) ~= round(128*(S*SCALE*log2(e)) + 128*(127-sigma))
EXP2_SIGMA = 0.0435
EXP2_A = 128.0 * SCALE * 1.4426950408889634
EXP2_B = 128.0 * (127.0 - EXP2_SIGMA)

import os as _os
ACT_SHARE = float(_os.environ.get("K_ACT_SHARE", "0.50"))  # exp tiles on ScalarE
PV_LAG = int(_os.environ.get("K_PV_LAG", "8"))  # steps PV trails S/exp
K_VT3_ACT = _os.environ.get("K_VT3_ACT", "0") == "1"  # vT3 copies on Act vs DVE
K_HEAD = _os.environ.get("K_HEAD", "serial")  # interleave | serial | kv2
K_FIXED = _os.environ.get("K_FIXED", "act2")  # mix | dve | act2  (non-exp copy placement)


def build_nc():
    nc = bacc.Bacc()

    x_d = nc.dram_tensor("x", [C, N], bf16, kind="ExternalInput")
    xq_d = nc.dram_tensor("xq", [C, NQ], bf16, kind="ExternalInput")
    wqT_d = nc.dram_tensor("wqT", [C, INNER], bf16, kind="ExternalInput")
    wkT_d = nc.dram_tensor("wkT", [C, INNER], bf16, kind="ExternalInput")
    wvT_d = nc.dram_tensor("wvT", [C, INNER], bf16, kind="ExternalInput")
    woT_d = nc.dram_tensor("woT", [INNER, C], bf16, kind="ExternalInput")
    eye_d = nc.dram_tensor("eye", [128, 128], bf16, kind="ExternalInput")
    biasT_d = nc.dram_tensor("biasT", [128, 2], f32, kind="ExternalInput")
    out_d = nc.dram_tensor("out", [C, NQ], f32, kind="ExternalOutput")
    import os
    _dbg = os.environ.get("KDBG", "0") == "1"
    if _dbg:
        dbg_k = nc.dram_tensor("dbg_k", [128, N], f32, kind="ExternalOutput")
        dbg_q = nc.dram_tensor("dbg_q", [128, NQ], f32, kind="ExternalOutput")
        dbg_v3 = nc.dram_tensor("dbg_v3", [128, NJT * HEADS * (DH + 1)], f32, kind="ExternalOutput")
        dbg_pt = nc.dram_tensor("dbg_pt", [128, 2 * QB], f32, kind="ExternalOutput")
        dbg_atb = nc.dram_tensor("dbg_atb", [128, 16 * DH], f32, kind="ExternalOutput")
        dbg_acc = nc.dram_tensor("dbg_acc", [128, 8 * (DH + 1)], f32, kind="ExternalOutput")
        dbg_a = nc.dram_tensor("dbg_a", [128, 512], f32, kind="ExternalOutput")

    with tile.TileContext(nc) as tc:
        import contextlib

        ctx = contextlib.ExitStack()
        with ctx:
            big = ctx.enter_context(tc.tile_pool(name="big", bufs=1))
            wkp = ctx.enter_context(tc.tile_pool(name="wkp", bufs=2))
            ptp = ctx.enter_context(tc.tile_pool(name="ptp", bufs=PV_LAG + 2))
            ps = ctx.enter_context(tc.tile_pool(name="ps", bufs=2, space="PSUM"))

            # ---- constants / weights ----
            wq_sb = big.tile([128, 2, INNER], bf16)   # [c_part, c_chunk, inner]
            wk_sb = big.tile([128, 2, INNER], bf16)
            wv_sb = big.tile([128, 2, INNER], bf16)
            wo_sb = big.tile([128, C], bf16)          # [inner, c]
            eye_sb = big.tile([128, 128], bf16)
            bias_sb = big.tile([128, 2], f32)

            # ---- activation DMA (chunked, interleaved with projections) ----
            x_sb = big.tile([128, 2, N], bf16)    # [c_part, c_chunk, n]
            xq_sb = big.tile([128, 2, NQ], bf16)

            k_sb = big.tile([128, N], f32r)       # [inner, n]
            q_sb = big.tile([128, NQ], f32r)      # [inner, nq]
            v_sb = big.tile([128, N], bf16)       # [inner, n]
            # v^T + ones col: [j0, (jtile, head), 33]; col 32 stays 1.0
            vT3 = big.tile([128, NJT * HEADS, DH + 1], bf16)
            nc.vector.memset(vT3[:, :, DH:DH + 1], 1.0)

            # ---- DMA priority order: xq -> wq/wk -> x chunk 0 -> the rest.
            # q projection runs first (small); k/v chunk work is emitted
            # lazily inside the attention stream so PE starts S^T as soon as
            # the first k tile lands ----
            for cc in range(2):
                nc.sync.dma_start(out=xq_sb[:, cc, :], in_=xq_d[128 * cc:128 * (cc + 1), :])
            for cc in range(2):
                nc.sync.dma_start(out=wq_sb[:, cc, :], in_=wqT_d[128 * cc:128 * (cc + 1), :])
                nc.sync.dma_start(out=wk_sb[:, cc, :], in_=wkT_d[128 * cc:128 * (cc + 1), :])
            for t in range(NQ // 512):
                qp = ps.tile([128, 2, 512], f32, tag="st", bufs=3, name="qp")[:, 0, :]
                for cc in range(2):
                    nc.tensor.matmul(
                        out=qp[:],
                        lhsT=wq_sb[:, cc, :],
                        rhs=xq_sb[:, cc, 512 * t:512 * (t + 1)],
                        start=(cc == 0), stop=(cc == 1),
                    )
                if K_FIXED in ("mix", "act2"):
                    nc.scalar.copy(out=q_sb[:, 512 * t:512 * (t + 1)], in_=qp[:])
                else:
                    nc.vector.tensor_copy(out=q_sb[:, 512 * t:512 * (t + 1)], in_=qp[:])

            def emit_x_dma(ch):
                c0 = 1024 * ch
                for cc in range(2):
                    nc.sync.dma_start(
                        out=x_sb[:, cc, c0:c0 + 1024],
                        in_=x_d[128 * cc:128 * (cc + 1), c0:c0 + 1024],
                    )

            def emit_k_tile(t):
                kp = ps.tile([128, 2, 512], f32, tag="st", bufs=3, name="kp")[:, 0, :]
                for cc in range(2):
                    nc.tensor.matmul(
                        out=kp[:],
                        lhsT=wk_sb[:, cc, :],
                        rhs=x_sb[:, cc, 512 * t:512 * (t + 1)],
                        start=(cc == 0), stop=(cc == 1),
                    )
                if K_FIXED == "act2" or (K_FIXED == "mix" and t % 2 == 0):
                    nc.scalar.copy(out=k_sb[:, 512 * t:512 * (t + 1)], in_=kp[:])
                else:
                    nc.vector.tensor_copy(out=k_sb[:, 512 * t:512 * (t + 1)], in_=kp[:])

            def emit_v_tile(t):
                # v projection for n-cols [512t, 512t+512), then transpose into
                # vT3 (full-128 transposes, no row-strip hazards)
                vp = ps.tile([128, 2, 512], f32, tag="st", bufs=3, name="vp")[:, 0, :]
                for cc in range(2):
                    nc.tensor.matmul(
                        out=vp[:],
                        lhsT=wv_sb[:, cc, :],
                        rhs=x_sb[:, cc, 512 * t:512 * (t + 1)],
                        start=(cc == 0), stop=(cc == 1),
                    )
                if K_FIXED == "act2":
                    nc.scalar.copy(out=v_sb[:, 512 * t:512 * (t + 1)], in_=vp[:])
                else:
                    nc.vector.tensor_copy(out=v_sb[:, 512 * t:512 * (t + 1)], in_=vp[:])
                vp2_t = ps.tile([128, 2, 512], f32, tag="st", bufs=3, name="vp2")
                vp2 = vp2_t[:, 0, :].bitcast(bf16).rearrange("p (a b) -> p a b", b=128)
                for j2 in range(4):
                    j = 4 * t + j2
                    nc.tensor.transpose(
                        out=vp2[:, j2:j2 + 1, :],
                        in_=v_sb[:, 128 * j:128 * (j + 1)],
                        identity=eye_sb[:],
                    )
                src = vp2[:, 0:4, :].rearrange("p j (h d) -> p (j h) d", d=DH)
                eng = nc.scalar.copy if K_VT3_ACT else nc.vector.tensor_copy
                eng(out=vT3[:, 16 * t:16 * (t + 1), 0:DH], in_=src)

            emit_x_dma(0)
            for cc in range(2):
                nc.sync.dma_start(out=wv_sb[:, cc, :], in_=wvT_d[128 * cc:128 * (cc + 1), :])
            nc.sync.dma_start(out=eye_sb[:], in_=eye_d[:])
            nc.sync.dma_start(out=wo_sb[:], in_=woT_d[:])
            nc.sync.dma_start(out=bias_sb[:], in_=biasT_d[:])
            if K_HEAD == "serial":
                for ch in range(1, 4):
                    emit_x_dma(ch)
                for t in range(8):
                    emit_k_tile(t)
                    emit_v_tile(t)

            # ---- attention: software-pipelined (qb, p, J) stream ----
            steps = [(qb, p, J) for qb in range(NQB) for p in range(2)
                     for J in range(NJT)]
            nsteps = len(steps)
            exp_err = 0.0
            pt_tiles = {}     # step idx -> pt tile
            acc_tiles = {}    # (qb, p) -> acc psum tile
            atb_tiles = {}    # qb -> normalized A^T sbuf tile
            pv_prev = None
            # scheduled tail work: emitted after the S/PV of the given step
            tail_pair = {}    # step idx -> (qb, p) whose PVs just finished
            tail_qb = {}      # step idx -> qb ready for transpose/proj/out

            for i in range(nsteps + PV_LAG + 5):
                # lazy k/v projection work: k tile t at step 2t, v tile t at
                # step 2t+1, next x chunk ahead of need
                if K_HEAD == "interleave" and i < 16:
                    t = i // 2
                    if i % 2 == 0:
                        if t % 2 == 1 and t // 2 + 1 < 4:
                            emit_x_dma(t // 2 + 1)
                        emit_k_tile(t)
                    else:
                        emit_v_tile(t)
                if K_HEAD == "kv2":
                    # k tile t at step 2t (S(J) needs tile J//4 by step J);
                    # v tile t at step 2t+16 (PV(J) needs it by step J+PV_LAG)
                    if i < 16 and i % 2 == 0:
                        t = i // 2
                        if t % 2 == 1 and t // 2 + 1 < 4:
                            emit_x_dma(t // 2 + 1)
                        emit_k_tile(t)
                    if 16 <= i < 32 and i % 2 == 0:
                        emit_v_tile((i - 16) // 2)

                # S^T + exp for step i
                if i < nsteps:
                    qb, p, J = steps[i]
                    q0 = QB * qb
                    if (qb, p) not in acc_tiles:
                        acc_t = ps.tile([128, 512], f32, tag="acc", bufs=2, name="acc")
                        acc_tiles[(qb, p)] = acc_t[:, 0:8 * (DH + 1)].rearrange(
                            "p (a b) -> p a b", b=DH + 1
                        )
                    st = ps.tile([128, 2, QB], f32, tag="st", bufs=3, name="st")
                    for hh in range(2):
                        h = 2 * p + hh
                        nc.tensor.matmul(
                            out=st[:, hh, :],
                            lhsT=k_sb[32 * h:32 * (h + 1), JT * J:JT * (J + 1)],
                            rhs=q_sb[32 * h:32 * (h + 1), q0:q0 + QB],
                            start=True, stop=True,
                            tile_position=(32 * h, 0),
                        )
                    pt = ptp.tile([128, 2, QB], bf16, tag="pt", name="pt")
                    pt_tiles[i] = pt
                    dump_pt = _dbg and i == 0
                    exp_err += ACT_SHARE
                    if exp_err >= 1.0:
                        exp_err -= 1.0
                        nc.scalar.activation(
                            out=pt[:], in_=st[:],
                            func=mybir.ActivationFunctionType.Exp,
                            scale=SCALE,
                        )
                    else:
                        nc.vector.tensor_scalar(
                            out=pt[:].bitcast(i16), in0=st[:],
                            scalar1=EXP2_A, scalar2=EXP2_B,
                            op0=mybir.AluOpType.mult,
                            op1=mybir.AluOpType.add,
                        )
                    if dump_pt:
                        dpt = wkp.tile([128, 2 * QB], f32, tag="dbgpt", name="dbgpt")
                        nc.vector.tensor_copy(out=dpt[:], in_=pt.rearrange("p a b -> p (a b)"))
                        nc.sync.dma_start(out=dbg_pt[:], in_=dpt[:])

                # PV for step i - PV_LAG
                j = i - PV_LAG
                if 0 <= j < nsteps:
                    qb, p, J = steps[j]
                    acc = acc_tiles[(qb, p)]
                    pt = pt_tiles.pop(j)
                    for hh in range(2):
                        h = 2 * p + hh
                        for qs in range(4):
                            # start=True resets has_written for the WHOLE psum
                            # bank (values persist), so only the first matmul
                            # into the bank may set it; later groups' J=0
                            # writes land on cleared bits and start fresh.
                            mm = nc.tensor.matmul(
                                out=acc[:, 2 * qs + hh, :],
                                lhsT=pt[:, hh, 128 * qs:128 * (qs + 1)],
                                rhs=vT3[:, HEADS * J + h, :],
                                start=(J == 0 and hh == 0 and qs == 0),
                                stop=(J == NJT - 1),
                                skip_group_check=True,
                            )
                            if pv_prev is not None:
                                add_dep_helper(mm.ins, pv_prev.ins, sync=False,
                                               reason="pv order")
                            pv_prev = mm
                    if J == NJT - 1:
                        tail_pair[i + 1] = (qb, p)
                        if p == 1:
                            tail_qb[i + (2 if qb == NQB - 1 else 4)] = qb

                # per-pair normalization as soon as a pair's PVs are done:
                # DVE copies acc->SBUF + reciprocal; GPSIMD scales (SBUF only)
                if i in tail_pair:
                    qb, p = tail_pair.pop(i)
                    acc = acc_tiles.pop((qb, p))
                    acc_sb = wkp.tile([128, 8, DH + 1], f32, tag="accsb", name="acc_sb")
                    if K_FIXED == "act2":
                        nc.scalar.copy(out=acc_sb[:], in_=acc[:])
                    else:
                        nc.vector.tensor_copy(out=acc_sb[:], in_=acc[:])
                    if _dbg and qb == 0 and p == 0:
                        nc.sync.dma_start(out=dbg_acc[:], in_=acc_sb.rearrange("p a b -> p (a b)"))
                    rcp_sb = wkp.tile([128, 8], f32, tag="rcp", name="rcp")
                    nc.vector.reciprocal(out=rcp_sb[:], in_=acc_sb[:, :, DH])
                    if qb not in atb_tiles:
                        atb_tiles[qb] = wkp.tile([128, 16, DH], bf16, tag="atb", name="atb")
                    atb = atb_tiles[qb]
                    for qs in range(4):
                        for hh in range(2):
                            nc.gpsimd.tensor_scalar_mul(
                                out=atb[:, 4 * qs + 2 * p + hh, :],
                                in0=acc_sb[:, 2 * qs + hh, 0:DH],
                                scalar1=rcp_sb[:, 2 * qs + hh:2 * qs + hh + 1],
                            )

                # per-qb finish: transpose A^T -> A, project, bias, DMA out
                if i in tail_qb:
                    qb = tail_qb.pop(i)
                    q0 = QB * qb
                    atb = atb_tiles.pop(qb)
                    a_ps_t = ps.tile([128, 2, 512], f32, tag="st", bufs=3, name="a_ps")
                    a_ps = a_ps_t[:, 0, :].bitcast(bf16).rearrange("p (a b) -> p a b", b=128)
                    atb_flat = atb.rearrange("p i d -> p (i d)")
                    for qs in range(4):
                        nc.tensor.transpose(
                            out=a_ps[:, qs:qs + 1, :],
                            in_=atb_flat[:, 128 * qs:128 * (qs + 1)],
                            identity=eye_sb[:],
                        )
                    a_sb = wkp.tile([128, 4, 128], bf16, tag="asb", name="a_sb")
                    if K_FIXED == "act2":
                        nc.scalar.copy(out=a_sb[:], in_=a_ps[:, 0:4, :])
                    else:
                        nc.vector.tensor_copy(out=a_sb[:], in_=a_ps[:, 0:4, :])
                    if _dbg and qb == 0:
                        datb = wkp.tile([128, 16 * DH], f32, tag="dbgatb", name="dbgatb")
                        nc.vector.tensor_copy(out=datb[:], in_=atb.rearrange("p a b -> p (a b)"))
                        nc.sync.dma_start(out=dbg_atb[:], in_=datb[:])
                        da = wkp.tile([128, 512], f32, tag="dbga", name="dbga")
                        nc.vector.tensor_copy(out=da[:], in_=a_sb.rearrange("p a b -> p (a b)"))
                        nc.sync.dma_start(out=dbg_a[:], in_=da[:])
                    a_flat = a_sb.rearrange("p a b -> p (a b)")
                    for cb in range(2):
                        op = ps.tile([128, 2, QB], f32, tag="st", bufs=3, name="op")[:, 0, :]
                        nc.tensor.matmul(
                            out=op[:],
                            lhsT=wo_sb[:, 128 * cb:128 * (cb + 1)],
                            rhs=a_flat[:],
                            start=True, stop=True,
                        )
                        ob = wkp.tile([128, QB], f32, tag="ob", name="ob")
                        if (K_FIXED == "mix") and cb == 0:
                            nc.scalar.add(out=ob[:], in_=op[:], add=bias_sb[:, cb:cb + 1])
                        elif K_FIXED == "act2" and cb == 0:
                            nc.scalar.add(out=ob[:], in_=op[:], add=bias_sb[:, cb:cb + 1])
                        else:
                            nc.vector.tensor_scalar_add(out=ob[:], in0=op[:], scalar1=bias_sb[:, cb:cb + 1])
                        nc.sync.dma_start(
                            out=out_d[128 * cb:128 * (cb + 1), q0:q0 + QB], in_=ob[:]
                        )

            # flush any remaining tails
            for i in sorted(list(tail_pair) + list(tail_qb)):
                assert False, "tails must be drained inside the loop"

            if _dbg:
                dk = wkp.tile([128, N], f32, tag="dbgk", name="dbgk")
                nc.vector.tensor_copy(out=dk[:], in_=k_sb[:])
                nc.sync.dma_start(out=dbg_k[:], in_=dk[:])
                dq = wkp.tile([128, NQ], f32, tag="dbgq", name="dbgq")
                nc.vector.tensor_copy(out=dq[:], in_=q_sb[:])
                nc.sync.dma_start(out=dbg_q[:], in_=dq[:])
                dv3 = wkp.tile([128, NJT * HEADS * (DH + 1)], f32, tag="dbgv", name="dbgv")
                nc.vector.tensor_copy(out=dv3[:], in_=vT3.rearrange("p a b -> p (a b)"))
                nc.sync.dma_start(out=dbg_v3[:], in_=dv3[:])



    nc.compile()
    return nc


_NC_CACHE = []


def _get_nc():
    if not _NC_CACHE:
        _NC_CACHE.append(build_nc())
    return _NC_CACHE[0]


def _make_in_maps(x, Wq, Wk, Wv, Wout, bout):
    import ml_dtypes

    bfl = ml_dtypes.bfloat16
    xf = np.ascontiguousarray(x.reshape(B, C, N)).astype(bfl)
    wqT = np.ascontiguousarray(np.asarray(Wq, dtype=np.float32).T.astype(bfl))
    wkT = np.ascontiguousarray(np.asarray(Wk, dtype=np.float32).T.astype(bfl))
    wvT = np.ascontiguousarray(np.asarray(Wv, dtype=np.float32).T.astype(bfl))
    woT = np.ascontiguousarray(np.asarray(Wout, dtype=np.float32).T.astype(bfl))
    eye = np.eye(128, dtype=bfl)
    biasT = np.ascontiguousarray(
        np.asarray(bout, dtype=np.float32).reshape(2, 128).T
    )
    in_maps = []
    for core in range(8):
        b, half = core // 2, core % 2
        q0 = half * NQ
        in_maps.append({
            "x": xf[b],
            "xq": np.ascontiguousarray(xf[b][:, q0:q0 + NQ]),
            "wqT": wqT, "wkT": wkT, "wvT": wvT,
            "woT": woT, "eye": eye, "biasT": biasT,
        })
    return in_maps


def kernel(x, Wq, Wk, Wv, Wout, bout):
    nc = _get_nc()
    in_maps = _make_in_maps(x, Wq, Wk, Wv, Wout, bout)
    res = run_bass_kernel_spmd(nc, in_maps, core_ids=list(range(8)))
    out = np.empty((B, C, N), dtype=np.float32)
    for core in range(8):
        b, half = core // 2, core % 2
        q0 = half * NQ
        out[b][:, q0:q0 + NQ] = res.results[core]["out"]
    return out.reshape(B, C, 64, 64)


# revision 25
# speedup vs baseline: 4.7897x; 1.0012x over previous
"""Trainium2 Bass kernel for 4-head spatial self-attention.

Computation (per batch b):
    xf = x[b] reshaped [C=256, n=4096]
    q/k/v = Wq/Wk/Wv @ xf            -> [128, n]   (rows = 4 heads x 32 dims)
    S_h   = (q_h^T k_h) * 32^-0.5    -> [n, n] per head
    P     = exp(S)  (softmax without max-subtraction: logits are O(10), safe)
    A_h   = P_h V_h^T / rowsum       -> [n, 32]
    out   = Wout @ A + bout          -> [C, n]

Sharding: 8 cores = 4 batches x 2 query-halves. Each core handles all 4 heads
for one batch and 2048 queries vs all 4096 keys; outputs are disjoint slices.

Perf notes (cost-model driven):
 - All matmul operands are float32r or bf16: 1 PE cycle per output free-row
   (fp32 costs 4).  float32r needs moving-free >= 256, so the small-free
   matmuls (PV, transposes) use bf16; q/k stay f32r for exact logits.
 - S^T is computed with keys on partitions (queries free) so exp(S^T) tiles
   feed PV directly as stationary operands.  The d=32 head contractions pack
   onto PE row strips via tile_position; concurrent strip matmuls must write
   different PSUM banks ([128, 2, 512] st tile, one bank per head).
 - PV is computed TRANSPOSED: A^T[128q, 33] += pt_slice.T @ [V^T | 1].  The
   moving operand is 33 wide (vs 512 the other way round).  Column 32 of the
   rhs is ones, so A^T col 32 accumulates the softmax denominator for free.
 - exp is split across ScalarE (true exp, PSUM->SBUF bf16) and DVE
   (Schraudolph exp2: one tensor_scalar mult+add writing int16 bits that
   reinterpret as bf16 ~= 2^y).
 - The (pair, J) stream is software-pipelined: PV lags S/exp by PV_LAG steps
   so the in-order PE queue never waits on the exp engines.
 - A^T normalization: DVE copies acc->SBUF + reciprocal of the denominator
   column; the 16 per-(head,qs) scale-multiplies run on otherwise-idle
   GPSIMD (SBUF only).  A^T is transposed back to [hd, q] with full-128
   PE transposes (bf16) and projected with Wout^T stationary.
"""

import numpy as np
import sys

for _p in ("/opt/trn_rl_repo", "/opt/pypackages"):
    if _p not in sys.path:
        sys.path.append(_p)

import concourse.bass as bass
import concourse.tile as tile
from concourse import bacc, mybir
from concourse.tile import add_dep_helper
from concourse.bass_utils import run_bass_kernel_spmd

f32 = mybir.dt.float32
f32r = mybir.dt.float32r
bf16 = mybir.dt.bfloat16
i16 = mybir.dt.int16

B = 4
C = 256
N = 4096          # h*w = 64*64 key positions
NQ = 2048         # queries per core (half batch)
HEADS = 4
DH = 32
INNER = 128
SCALE = DH ** -0.5

QB = 512          # query block (free dim of S^T tiles)
NQB = NQ // QB    # 4
JT = 128          # key tile (partition dim of S^T tiles)
NJT = N // JT     # 32

# Schraudolph exp2 constants for the bf16 bit pattern:
#   bf16_bits(e^(S*SCALE)) ~= round(128*(S*SCALE*log2(e)) + 128*(127-sigma))
EXP2_SIGMA = 0.0435
EXP2_A = 128.0 * SCALE * 1.4426950408889634
EXP2_B = 128.0 * (127.0 - EXP2_SIGMA)

ACT_SHARE = 0.50  # fraction of exp tiles on ScalarE (rest: DVE Schraudolph)
PV_LAG = 10       # steps PV trails S/exp in the software pipeline
K_VT3_ACT = False  # vT3 copies on DVE
K_HEAD = "serial"  # proj phase before the attention stream (engines idle there)
K_FIXED = "act2"   # k/q/v/acc/a copies on ScalarE; balances the DVE exp load


def build_nc():
    nc = bacc.Bacc()

    x_d = nc.dram_tensor("x", [C, N], bf16, kind="ExternalInput")
    xq_d = nc.dram_tensor("xq", [C, NQ], bf16, kind="ExternalInput")
    wqT_d = nc.dram_tensor("wqT", [C, INNER], bf16, kind="ExternalInput")
    wkT_d = nc.dram_tensor("wkT", [C, INNER], bf16, kind="ExternalInput")
    wvT_d = nc.dram_tensor("wvT", [C, INNER], bf16, kind="ExternalInput")
    woT_d = nc.dram_tensor("woT", [INNER, C], bf16, kind="ExternalInput")
    eye_d = nc.dram_tensor("eye", [128, 128], bf16, kind="ExternalInput")
    biasT_d = nc.dram_tensor("biasT", [128, 2], f32, kind="ExternalInput")
    out_d = nc.dram_tensor("out", [C, NQ], f32, kind="ExternalOutput")
    import os
    _dbg = os.environ.get("KDBG", "0") == "1"
    if _dbg:
        dbg_k = nc.dram_tensor("dbg_k", [128, N], f32, kind="ExternalOutput")
        dbg_q = nc.dram_tensor("dbg_q", [128, NQ], f32, kind="ExternalOutput")
        dbg_v3 = nc.dram_tensor("dbg_v3", [128, NJT * HEADS * (DH + 1)], f32, kind="ExternalOutput")
        dbg_pt = nc.dram_tensor("dbg_pt", [128, 2 * QB], f32, kind="ExternalOutput")
        dbg_atb = nc.dram_tensor("dbg_atb", [128, 16 * DH], f32, kind="ExternalOutput")
        dbg_acc = nc.dram_tensor("dbg_acc", [128, 8 * (DH + 1)], f32, kind="ExternalOutput")
        dbg_a = nc.dram_tensor("dbg_a", [128, 512], f32, kind="ExternalOutput")

    with tile.TileContext(nc) as tc:
        import contextlib

        ctx = contextlib.ExitStack()
        with ctx:
            big = ctx.enter_context(tc.tile_pool(name="big", bufs=1))
            wkp = ctx.enter_context(tc.tile_pool(name="wkp", bufs=2))
            ptp = ctx.enter_context(tc.tile_pool(name="ptp", bufs=PV_LAG + 2))
            ps = ctx.enter_context(tc.tile_pool(name="ps", bufs=2, space="PSUM"))

            # ---- constants / weights ----
            wq_sb = big.tile([128, 2, INNER], bf16)   # [c_part, c_chunk, inner]
            wk_sb = big.tile([128, 2, INNER], bf16)
            wv_sb = big.tile([128, 2, INNER], bf16)
            wo_sb = big.tile([128, C], bf16)          # [inner, c]
            eye_sb = big.tile([128, 128], bf16)
            bias_sb = big.tile([128, 2], f32)

            # ---- activation DMA (chunked, interleaved with projections) ----
            x_sb = big.tile([128, 2, N], bf16)    # [c_part, c_chunk, n]
            xq_sb = big.tile([128, 2, NQ], bf16)

            k_sb = big.tile([128, N], f32r)       # [inner, n]
            q_sb = big.tile([128, NQ], f32r)      # [inner, nq]
            v_sb = big.tile([128, N], bf16)       # [inner, n]
            # v^T + ones col: [j0, (jtile, head), 33]; col 32 stays 1.0
            vT3 = big.tile([128, NJT * HEADS, DH + 1], bf16)
            nc.vector.memset(vT3[:, :, DH:DH + 1], 1.0)

            # ---- DMA priority order: xq -> wq/wk -> x chunk 0 -> the rest.
            # q projection runs first (small); k/v chunk work is emitted
            # lazily inside the attention stream so PE starts S^T as soon as
            # the first k tile lands ----
            for cc in range(2):
                nc.sync.dma_start(out=xq_sb[:, cc, :], in_=xq_d[128 * cc:128 * (cc + 1), :])
            for cc in range(2):
                nc.sync.dma_start(out=wq_sb[:, cc, :], in_=wqT_d[128 * cc:128 * (cc + 1), :])
                nc.sync.dma_start(out=wk_sb[:, cc, :], in_=wkT_d[128 * cc:128 * (cc + 1), :])
            for t in range(NQ // 512):
                qp = ps.tile([128, 2, 512], f32, tag="st", bufs=3, name="qp")[:, 0, :]
                for cc in range(2):
                    nc.tensor.matmul(
                        out=qp[:],
                        lhsT=wq_sb[:, cc, :],
                        rhs=xq_sb[:, cc, 512 * t:512 * (t + 1)],
                        start=(cc == 0), stop=(cc == 1),
                    )
                if K_FIXED in ("mix", "act2"):
                    nc.scalar.copy(out=q_sb[:, 512 * t:512 * (t + 1)], in_=qp[:])
                else:
                    nc.vector.tensor_copy(out=q_sb[:, 512 * t:512 * (t + 1)], in_=qp[:])

            def emit_x_dma(ch):
                c0 = 1024 * ch
                for cc in range(2):
                    nc.sync.dma_start(
                        out=x_sb[:, cc, c0:c0 + 1024],
                        in_=x_d[128 * cc:128 * (cc + 1), c0:c0 + 1024],
                    )

            def emit_k_tile(t):
                kp = ps.tile([128, 2, 512], f32, tag="st", bufs=3, name="kp")[:, 0, :]
                for cc in range(2):
                    nc.tensor.matmul(
                        out=kp[:],
                        lhsT=wk_sb[:, cc, :],
                        rhs=x_sb[:, cc, 512 * t:512 * (t + 1)],
                        start=(cc == 0), stop=(cc == 1),
                    )
                if K_FIXED == "act2" or (K_FIXED == "mix" and t % 2 == 0):
                    nc.scalar.copy(out=k_sb[:, 512 * t:512 * (t + 1)], in_=kp[:])
                else:
                    nc.vector.tensor_copy(out=k_sb[:, 512 * t:512 * (t + 1)], in_=kp[:])

            def emit_v_tile(t):
                # v projection for n-cols [512t, 512t+512), then transpose into
                # vT3 (full-128 transposes, no row-strip hazards)
                vp = ps.tile([128, 2, 512], f32, tag="st", bufs=3, name="vp")[:, 0, :]
                for cc in range(2):
                    nc.tensor.matmul(
                        out=vp[:],
                        lhsT=wv_sb[:, cc, :],
                        rhs=x_sb[:, cc, 512 * t:512 * (t + 1)],
                        start=(cc == 0), stop=(cc == 1),
                    )
                if K_FIXED == "act2":
                    nc.scalar.copy(out=v_sb[:, 512 * t:512 * (t + 1)], in_=vp[:])
                else:
                    nc.vector.tensor_copy(out=v_sb[:, 512 * t:512 * (t + 1)], in_=vp[:])
                vp2_t = ps.tile([128, 2, 512], f32, tag="st", bufs=3, name="vp2")
                vp2 = vp2_t[:, 0, :].bitcast(bf16).rearrange("p (a b) -> p a b", b=128)
                for j2 in range(4):
                    j = 4 * t + j2
                    nc.tensor.transpose(
                        out=vp2[:, j2:j2 + 1, :],
                        in_=v_sb[:, 128 * j:128 * (j + 1)],
                        identity=eye_sb[:],
                    )
                src = vp2[:, 0:4, :].rearrange("p j (h d) -> p (j h) d", d=DH)
                eng = nc.scalar.copy if K_VT3_ACT else nc.vector.tensor_copy
                eng(out=vT3[:, 16 * t:16 * (t + 1), 0:DH], in_=src)

            emit_x_dma(0)
            for cc in range(2):
                nc.sync.dma_start(out=wv_sb[:, cc, :], in_=wvT_d[128 * cc:128 * (cc + 1), :])
            nc.sync.dma_start(out=eye_sb[:], in_=eye_d[:])
            nc.sync.dma_start(out=wo_sb[:], in_=woT_d[:])
            nc.sync.dma_start(out=bias_sb[:], in_=biasT_d[:])
            if K_HEAD == "serial":
                for ch in range(1, 4):
                    emit_x_dma(ch)
                for t in range(8):
                    emit_k_tile(t)
                    emit_v_tile(t)

            # ---- attention: software-pipelined (qb, p, J) stream ----
            steps = [(qb, p, J) for qb in range(NQB) for p in range(2)
                     for J in range(NJT)]
            nsteps = len(steps)
            exp_err = 0.0
            pt_tiles = {}     # step idx -> pt tile
            acc_tiles = {}    # (qb, p) -> acc psum tile
            atb_tiles = {}    # qb -> normalized A^T sbuf tile
            pv_prev = None
            # scheduled tail work: emitted after the S/PV of the given step
            tail_pair = {}    # step idx -> (qb, p) whose PVs just finished
            tail_qb = {}      # step idx -> qb ready for transpose/proj/out

            for i in range(nsteps + PV_LAG + 5):
                # lazy k/v projection work: k tile t at step 2t, v tile t at
                # step 2t+1, next x chunk ahead of need
                if K_HEAD == "interleave" and i < 16:
                    t = i // 2
                    if i % 2 == 0:
                        if t % 2 == 1 and t // 2 + 1 < 4:
                            emit_x_dma(t // 2 + 1)
                        emit_k_tile(t)
                    else:
                        emit_v_tile(t)
                if K_HEAD == "kv2":
                    # k tile t at step 2t (S(J) needs tile J//4 by step J);
                    # v tile t at step 2t+16 (PV(J) needs it by step J+PV_LAG)
                    if i < 16 and i % 2 == 0:
                        t = i // 2
                        if t % 2 == 1 and t // 2 + 1 < 4:
                            emit_x_dma(t // 2 + 1)
                        emit_k_tile(t)
                    if 16 <= i < 32 and i % 2 == 0:
                        emit_v_tile((i - 16) // 2)

                # S^T + exp for step i
                if i < nsteps:
                    qb, p, J = steps[i]
                    q0 = QB * qb
                    if (qb, p) not in acc_tiles:
                        acc_t = ps.tile([128, 512], f32, tag="acc", bufs=2, name="acc")
                        acc_tiles[(qb, p)] = acc_t[:, 0:8 * (DH + 1)].rearrange(
                            "p (a b) -> p a b", b=DH + 1
                        )
                    st = ps.tile([128, 2, QB], f32, tag="st", bufs=3, name="st")
                    for hh in range(2):
                        h = 2 * p + hh
                        nc.tensor.matmul(
                            out=st[:, hh, :],
                            lhsT=k_sb[32 * h:32 * (h + 1), JT * J:JT * (J + 1)],
                            rhs=q_sb[32 * h:32 * (h + 1), q0:q0 + QB],
                            start=True, stop=True,
                            tile_position=(32 * h, 0),
                        )
                    pt = ptp.tile([128, 2, QB], bf16, tag="pt", name="pt")
                    pt_tiles[i] = pt
                    dump_pt = _dbg and i == 0
                    exp_err += ACT_SHARE
                    if exp_err >= 1.0:
                        exp_err -= 1.0
                        nc.scalar.activation(
                            out=pt[:], in_=st[:],
                            func=mybir.ActivationFunctionType.Exp,
                            scale=SCALE,
                        )
                    else:
                        nc.vector.tensor_scalar(
                            out=pt[:].bitcast(i16), in0=st[:],
                            scalar1=EXP2_A, scalar2=EXP2_B,
                            op0=mybir.AluOpType.mult,
                            op1=mybir.AluOpType.add,
                        )
                    if dump_pt:
                        dpt = wkp.tile([128, 2 * QB], f32, tag="dbgpt", name="dbgpt")
                        nc.vector.tensor_copy(out=dpt[:], in_=pt.rearrange("p a b -> p (a b)"))
                        nc.sync.dma_start(out=dbg_pt[:], in_=dpt[:])

                # PV for step i - PV_LAG
                j = i - PV_LAG
                if 0 <= j < nsteps:
                    qb, p, J = steps[j]
                    acc = acc_tiles[(qb, p)]
                    pt = pt_tiles.pop(j)
                    for hh in range(2):
                        h = 2 * p + hh
                        for qs in range(4):
                            # start=True resets has_written for the WHOLE psum
                            # bank (values persist), so only the first matmul
                            # into the bank may set it; later groups' J=0
                            # writes land on cleared bits and start fresh.
                            mm = nc.tensor.matmul(
                                out=acc[:, 2 * qs + hh, :],
                                lhsT=pt[:, hh, 128 * qs:128 * (qs + 1)],
                                rhs=vT3[:, HEADS * J + h, :],
                                start=(J == 0 and hh == 0 and qs == 0),
                                stop=(J == NJT - 1),
                                skip_group_check=True,
                            )
                            if pv_prev is not None:
                                add_dep_helper(mm.ins, pv_prev.ins, sync=False,
                                               reason="pv order")
                            pv_prev = mm
                    if J == NJT - 1:
                        tail_pair[i + 1] = (qb, p)
                        if p == 1:
                            tail_qb[i + (2 if qb == NQB - 1 else 4)] = qb

                # per-pair normalization as soon as a pair's PVs are done:
                # DVE copies acc->SBUF + reciprocal; GPSIMD scales (SBUF only)
                if i in tail_pair:
                    qb, p = tail_pair.pop(i)
                    acc = acc_tiles.pop((qb, p))
                    acc_sb = wkp.tile([128, 8, DH + 1], f32, tag="accsb", name="acc_sb")
                    if K_FIXED == "act2":
                        nc.scalar.copy(out=acc_sb[:], in_=acc[:])
                    else:
                        nc.vector.tensor_copy(out=acc_sb[:], in_=acc[:])
                    if _dbg and qb == 0 and p == 0:
                        nc.sync.dma_start(out=dbg_acc[:], in_=acc_sb.rearrange("p a b -> p (a b)"))
                    rcp_sb = wkp.tile([128, 8], f32, tag="rcp", name="rcp")
                    nc.vector.reciprocal(out=rcp_sb[:], in_=acc_sb[:, :, DH])
                    if qb not in atb_tiles:
                        atb_tiles[qb] = wkp.tile([128, 16, DH], bf16, tag="atb", name="atb")
                    atb = atb_tiles[qb]
                    for qs in range(4):
                        for hh in range(2):
                            nc.gpsimd.tensor_scalar_mul(
                                out=atb[:, 4 * qs + 2 * p + hh, :],
                                in0=acc_sb[:, 2 * qs + hh, 0:DH],
                                scalar1=rcp_sb[:, 2 * qs + hh:2 * qs + hh + 1],
                            )

                # per-qb finish: transpose A^T -> A, project, bias, DMA out
                if i in tail_qb:
                    qb = tail_qb.pop(i)
                    q0 = QB * qb
                    atb = atb_tiles.pop(qb)
                    a_ps_t = ps.tile([128, 2, 512], f32, tag="st", bufs=3, name="a_ps")
                    a_ps = a_ps_t[:, 0, :].bitcast(bf16).rearrange("p (a b) -> p a b", b=128)
                    atb_flat = atb.rearrange("p i d -> p (i d)")
                    for qs in range(4):
                        nc.tensor.transpose(
                            out=a_ps[:, qs:qs + 1, :],
                            in_=atb_flat[:, 128 * qs:128 * (qs + 1)],
                            identity=eye_sb[:],
                        )
                    a_sb = wkp.tile([128, 4, 128], bf16, tag="asb", name="a_sb")
                    if K_FIXED == "act2":
                        nc.scalar.copy(out=a_sb[:], in_=a_ps[:, 0:4, :])
                    else:
                        nc.vector.tensor_copy(out=a_sb[:], in_=a_ps[:, 0:4, :])
                    if _dbg and qb == 0:
                        datb = wkp.tile([128, 16 * DH], f32, tag="dbgatb", name="dbgatb")
                        nc.vector.tensor_copy(out=datb[:], in_=atb.rearrange("p a b -> p (a b)"))
                        nc.sync.dma_start(out=dbg_atb[:], in_=datb[:])
                        da = wkp.tile([128, 512], f32, tag="dbga", name="dbga")
                        nc.vector.tensor_copy(out=da[:], in_=a_sb.rearrange("p a b -> p (a b)"))
                        nc.sync.dma_start(out=dbg_a[:], in_=da[:])
                    a_flat = a_sb.rearrange("p a b -> p (a b)")
                    for cb in range(2):
                        op = ps.tile([128, 2, QB], f32, tag="st", bufs=3, name="op")[:, 0, :]
                        nc.tensor.matmul(
                            out=op[:],
                            lhsT=wo_sb[:, 128 * cb:128 * (cb + 1)],
                            rhs=a_flat[:],
                            start=True, stop=True,
                        )
                        ob = wkp.tile([128, QB], f32, tag="ob", name="ob")
                        if (K_FIXED == "mix") and cb == 0:
                            nc.scalar.add(out=ob[:], in_=op[:], add=bias_sb[:, cb:cb + 1])
                        elif K_FIXED == "act2" and cb == 0:
                            nc.scalar.add(out=ob[:], in_=op[:], add=bias_sb[:, cb:cb + 1])
                        else:
                            nc.vector.tensor_scalar_add(out=ob[:], in0=op[:], scalar1=bias_sb[:, cb:cb + 1])
                        nc.sync.dma_start(
                            out=out_d[128 * cb:128 * (cb + 1), q0:q0 + QB], in_=ob[:]
                        )

            # flush any remaining tails
            for i in sorted(list(tail_pair) + list(tail_qb)):
                assert False, "tails must be drained inside the loop"

            if _dbg:
                dk = wkp.tile([128, N], f32, tag="dbgk", name="dbgk")
                nc.vector.tensor_copy(out=dk[:], in_=k_sb[:])
                nc.sync.dma_start(out=dbg_k[:], in_=dk[:])
                dq = wkp.tile([128, NQ], f32, tag="dbgq", name="dbgq")
                nc.vector.tensor_copy(out=dq[:], in_=q_sb[:])
                nc.sync.dma_start(out=dbg_q[:], in_=dq[:])
                dv3 = wkp.tile([128, NJT * HEADS * (DH + 1)], f32, tag="dbgv", name="dbgv")
                nc.vector.tensor_copy(out=dv3[:], in_=vT3.rearrange("p a b -> p (a b)"))
                nc.sync.dma_start(out=dbg_v3[:], in_=dv3[:])



    nc.compile()
    return nc


_NC_CACHE = []


def _get_nc():
    if not _NC_CACHE:
        _NC_CACHE.append(build_nc())
    return _NC_CACHE[0]


def _make_in_maps(x, Wq, Wk, Wv, Wout, bout):
    import ml_dtypes

    bfl = ml_dtypes.bfloat16
    xf = np.ascontiguousarray(x.reshape(B, C, N)).astype(bfl)
    wqT = np.ascontiguousarray(np.asarray(Wq, dtype=np.float32).T.astype(bfl))
    wkT = np.ascontiguousarray(np.asarray(Wk, dtype=np.float32).T.astype(bfl))
    wvT = np.ascontiguousarray(np.asarray(Wv, dtype=np.float32).T.astype(bfl))
    woT = np.ascontiguousarray(np.asarray(Wout, dtype=np.float32).T.astype(bfl))
    eye = np.eye(128, dtype=bfl)
    biasT = np.ascontiguousarray(
        np.asarray(bout, dtype=np.float32).reshape(2, 128).T
    )
    in_maps = []
    for core in range(8):
        b, half = core // 2, core % 2
        q0 = half * NQ
        in_maps.append({
            "x": xf[b],
            "xq": np.ascontiguousarray(xf[b][:, q0:q0 + NQ]),
            "wqT": wqT, "wkT": wkT, "wvT": wvT,
            "woT": woT, "eye": eye, "biasT": biasT,
        })
    return in_maps


def kernel(x, Wq, Wk, Wv, Wout, bout):
    nc = _get_nc()
    in_maps = _make_in_maps(x, Wq, Wk, Wv, Wout, bout)
    res = run_bass_kernel_spmd(nc, in_maps, core_ids=list(range(8)))
    out = np.empty((B, C, N), dtype=np.float32)
    for core in range(8):
        b, half = core // 2, core % 2
        q0 = half * NQ
        out[b][:, q0:q0 + NQ] = res.results[core]["out"]
    return out.reshape(B, C, 64, 64)


# revision 28
# speedup vs baseline: 4.7933x; 1.0007x over previous
"""Trainium2 Bass kernel for 4-head spatial self-attention.

Computation (per batch b):
    xf = x[b] reshaped [C=256, n=4096]
    q/k/v = Wq/Wk/Wv @ xf            -> [128, n]   (rows = 4 heads x 32 dims)
    S_h   = (q_h^T k_h) * 32^-0.5    -> [n, n] per head
    P     = exp(S)  (softmax without max-subtraction: logits are O(10), safe)
    A_h   = P_h V_h^T / rowsum       -> [n, 32]
    out   = Wout @ A + bout          -> [C, n]

Sharding: 8 cores = 4 batches x 2 query-halves. Each core handles all 4 heads
for one batch and 2048 queries vs all 4096 keys; outputs are disjoint slices.

Perf notes (cost-model driven):
 - All matmul operands are float32r or bf16: 1 PE cycle per output free-row
   (fp32 costs 4).  float32r needs moving-free >= 256, so the small-free
   matmuls (PV, transposes) use bf16; q/k stay f32r for exact logits.
 - S^T is computed with keys on partitions (queries free) so exp(S^T) tiles
   feed PV directly as stationary operands.  The d=32 head contractions pack
   onto PE row strips via tile_position; concurrent strip matmuls must write
   different PSUM banks ([128, 2, 512] st tile, one bank per head).
 - PV is computed TRANSPOSED: A^T[128q, 33] += pt_slice.T @ [V^T | 1].  The
   moving operand is 33 wide (vs 512 the other way round).  Column 32 of the
   rhs is ones, so A^T col 32 accumulates the softmax denominator for free.
 - exp is split across ScalarE (true exp, PSUM->SBUF bf16) and DVE
   (Schraudolph exp2: one tensor_scalar mult+add writing int16 bits that
   reinterpret as bf16 ~= 2^y).
 - The (pair, J) stream is software-pipelined: PV lags S/exp by PV_LAG steps
   so the in-order PE queue never waits on the exp engines.
 - A^T normalization: DVE copies acc->SBUF + reciprocal of the denominator
   column; the 16 per-(head,qs) scale-multiplies run on otherwise-idle
   GPSIMD (SBUF only).  A^T is transposed back to [hd, q] with full-128
   PE transposes (bf16) and projected with Wout^T stationary.
"""

import numpy as np
import sys

for _p in ("/opt/trn_rl_repo", "/opt/pypackages"):
    if _p not in sys.path:
        sys.path.append(_p)

import concourse.bass as bass
import concourse.tile as tile
from concourse import bacc, mybir
from concourse.tile import add_dep_helper
from concourse.bass_utils import run_bass_kernel_spmd

f32 = mybir.dt.float32
f32r = mybir.dt.float32r
bf16 = mybir.dt.bfloat16
i16 = mybir.dt.int16

B = 4
C = 256
N = 4096          # h*w = 64*64 key positions
NQ = 2048         # queries per core (half batch)
HEADS = 4
DH = 32
INNER = 128
SCALE = DH ** -0.5

QB = 512          # query block (free dim of S^T tiles)
NQB = NQ // QB    # 4
JT = 128          # key tile (partition dim of S^T tiles)
NJT = N // JT     # 32

# Schraudolph exp2 constants for the bf16 bit pattern:
#   bf16_bits(e^(S*SCALE)) ~= round(128*(S*SCALE*log2(e)) + 128*(127-sigma))
EXP2_SIGMA = 0.0435
EXP2_A = 128.0 * SCALE * 1.4426950408889634
EXP2_B = 128.0 * (127.0 - EXP2_SIGMA)

ACT_SHARE = 0.50  # fraction of exp tiles on ScalarE (rest: DVE Schraudolph)
PV_LAG = 10       # steps PV trails S/exp in the software pipeline
K_VT3_ACT = False  # vT3 copies on DVE
K_HEAD = "serial"  # proj phase before the attention stream (engines idle there)
K_FIXED = "act2"   # k/q/v/acc/a copies on ScalarE; balances the DVE exp load


def build_nc():
    nc = bacc.Bacc()

    x_d = nc.dram_tensor("x", [C, N], bf16, kind="ExternalInput")
    xq_d = nc.dram_tensor("xq", [C, NQ], bf16, kind="ExternalInput")
    wqkv_d = nc.dram_tensor("wqkv", [C, 3 * INNER], bf16, kind="ExternalInput")
    eyewo_d = nc.dram_tensor("eyewo", [128, 128 + C], bf16, kind="ExternalInput")
    biasT_d = nc.dram_tensor("biasT", [128, 2], f32, kind="ExternalInput")
    out_d = nc.dram_tensor("out", [C, NQ], f32, kind="ExternalOutput")
    import os
    _dbg = os.environ.get("KDBG", "0") == "1"
    if _dbg:
        dbg_k = nc.dram_tensor("dbg_k", [128, N], f32, kind="ExternalOutput")
        dbg_q = nc.dram_tensor("dbg_q", [128, NQ], f32, kind="ExternalOutput")
        dbg_v3 = nc.dram_tensor("dbg_v3", [128, NJT * HEADS * (DH + 1)], f32, kind="ExternalOutput")
        dbg_pt = nc.dram_tensor("dbg_pt", [128, 2 * QB], f32, kind="ExternalOutput")
        dbg_atb = nc.dram_tensor("dbg_atb", [128, 16 * DH], f32, kind="ExternalOutput")
        dbg_acc = nc.dram_tensor("dbg_acc", [128, 8 * (DH + 1)], f32, kind="ExternalOutput")
        dbg_a = nc.dram_tensor("dbg_a", [128, 512], f32, kind="ExternalOutput")

    with tile.TileContext(nc) as tc:
        import contextlib

        ctx = contextlib.ExitStack()
        with ctx:
            big = ctx.enter_context(tc.tile_pool(name="big", bufs=1))
            wkp = ctx.enter_context(tc.tile_pool(name="wkp", bufs=2))
            ptp = ctx.enter_context(tc.tile_pool(name="ptp", bufs=PV_LAG + 2))
            ps = ctx.enter_context(tc.tile_pool(name="ps", bufs=2, space="PSUM"))

            # ---- constants / weights (packed to minimize DMA count) ----
            wqkv_sb = big.tile([128, 2, 3 * INNER], bf16)  # [c_part, cc, (q|k|v)]
            eyewo_sb = big.tile([128, 128 + C], bf16)      # [inner, (eye|woT)]
            bias_sb = big.tile([128, 2], f32)
            wq_sb = wqkv_sb[:, :, 0:INNER]
            wk_sb = wqkv_sb[:, :, INNER:2 * INNER]
            wv_sb = wqkv_sb[:, :, 2 * INNER:3 * INNER]
            eye_sb = eyewo_sb[:, 0:128]
            wo_sb = eyewo_sb[:, 128:128 + C]

            # ---- activation DMA (chunked, interleaved with projections) ----
            x_sb = big.tile([128, 2, N], bf16)    # [c_part, c_chunk, n]
            xq_sb = big.tile([128, 2, NQ], bf16)

            k_sb = big.tile([128, N], f32r)       # [inner, n]
            q_sb = big.tile([128, NQ], f32r)      # [inner, nq]
            v_sb = big.tile([128, N], bf16)       # [inner, n]
            # v^T + ones col: [j0, (jtile, head), 33]; col 32 stays 1.0
            vT3 = big.tile([128, NJT * HEADS, DH + 1], bf16)
            nc.vector.memset(vT3[:, :, DH:DH + 1], 1.0)

            # ---- DMA priority order: weights -> x chunk 0 -> xq -> x chunk 1
            # -> eye/wo/bias.  k projection starts as soon as chunk 0 lands;
            # q follows xq; v waits for eye anyway. ----
            def emit_x_dma(ch):
                c0 = 1024 * ch
                for cc in range(2):
                    nc.sync.dma_start(
                        out=x_sb[:, cc, c0:c0 + 1024],
                        in_=x_d[128 * cc:128 * (cc + 1), c0:c0 + 1024],
                    )

            for cc in range(2):
                nc.sync.dma_start(out=xq_sb[:, cc, :], in_=xq_d[128 * cc:128 * (cc + 1), :])
            for cc in range(2):
                nc.sync.dma_start(out=wqkv_sb[:, cc, :], in_=wqkv_d[128 * cc:128 * (cc + 1), :])
            emit_x_dma(0)
            nc.sync.dma_start(out=eyewo_sb[:], in_=eyewo_d[:])
            nc.sync.dma_start(out=bias_sb[:], in_=biasT_d[:])
            for ch in range(1, 4):
                emit_x_dma(ch)

            for t in range(NQ // 512):
                qp = ps.tile([128, 2, 512], f32, tag="st", bufs=3, name="qp")[:, 0, :]
                for cc in range(2):
                    nc.tensor.matmul(
                        out=qp[:],
                        lhsT=wq_sb[:, cc, :],
                        rhs=xq_sb[:, cc, 512 * t:512 * (t + 1)],
                        start=(cc == 0), stop=(cc == 1),
                    )
                if K_FIXED in ("mix", "act2"):
                    nc.scalar.copy(out=q_sb[:, 512 * t:512 * (t + 1)], in_=qp[:])
                else:
                    nc.vector.tensor_copy(out=q_sb[:, 512 * t:512 * (t + 1)], in_=qp[:])

            def emit_k_tile(t):
                kp = ps.tile([128, 2, 512], f32, tag="st", bufs=3, name="kp")[:, 0, :]
                for cc in range(2):
                    nc.tensor.matmul(
                        out=kp[:],
                        lhsT=wk_sb[:, cc, :],
                        rhs=x_sb[:, cc, 512 * t:512 * (t + 1)],
                        start=(cc == 0), stop=(cc == 1),
                    )
                if K_FIXED == "act2" or (K_FIXED == "mix" and t % 2 == 0):
                    nc.scalar.copy(out=k_sb[:, 512 * t:512 * (t + 1)], in_=kp[:])
                else:
                    nc.vector.tensor_copy(out=k_sb[:, 512 * t:512 * (t + 1)], in_=kp[:])

            def emit_v_tile(t):
                # v projection for n-cols [512t, 512t+512), then transpose into
                # vT3 (full-128 transposes, no row-strip hazards)
                vp = ps.tile([128, 2, 512], f32, tag="st", bufs=3, name="vp")[:, 0, :]
                for cc in range(2):
                    nc.tensor.matmul(
                        out=vp[:],
                        lhsT=wv_sb[:, cc, :],
                        rhs=x_sb[:, cc, 512 * t:512 * (t + 1)],
                        start=(cc == 0), stop=(cc == 1),
                    )
                if K_FIXED == "act2":
                    nc.scalar.copy(out=v_sb[:, 512 * t:512 * (t + 1)], in_=vp[:])
                else:
                    nc.vector.tensor_copy(out=v_sb[:, 512 * t:512 * (t + 1)], in_=vp[:])
                vp2_t = ps.tile([128, 2, 512], f32, tag="st", bufs=3, name="vp2")
                vp2 = vp2_t[:, 0, :].bitcast(bf16).rearrange("p (a b) -> p a b", b=128)
                for j2 in range(4):
                    j = 4 * t + j2
                    nc.tensor.transpose(
                        out=vp2[:, j2:j2 + 1, :],
                        in_=v_sb[:, 128 * j:128 * (j + 1)],
                        identity=eye_sb[:],
                    )
                src = vp2[:, 0:4, :].rearrange("p j (h d) -> p (j h) d", d=DH)
                eng = nc.scalar.copy if K_VT3_ACT else nc.vector.tensor_copy
                eng(out=vT3[:, 16 * t:16 * (t + 1), 0:DH], in_=src)

            if K_HEAD == "serial":
                for t in range(8):
                    emit_k_tile(t)
                    emit_v_tile(t)

            # ---- attention: software-pipelined (qb, p, J) stream ----
            steps = [(qb, p, J) for qb in range(NQB) for p in range(2)
                     for J in range(NJT)]
            nsteps = len(steps)
            exp_err = 0.0
            pt_tiles = {}     # step idx -> pt tile
            acc_tiles = {}    # (qb, p) -> acc psum tile
            atb_tiles = {}    # qb -> normalized A^T sbuf tile
            pv_prev = None
            # scheduled tail work: emitted after the S/PV of the given step
            tail_pair = {}    # step idx -> (qb, p) whose PVs just finished
            tail_qb = {}      # step idx -> qb ready for transpose/proj/out

            for i in range(nsteps + PV_LAG + 5):
                # lazy k/v projection work: k tile t at step 2t, v tile t at
                # step 2t+1, next x chunk ahead of need

                # S^T + exp for step i
                if i < nsteps:
                    qb, p, J = steps[i]
                    q0 = QB * qb
                    if (qb, p) not in acc_tiles:
                        acc_t = ps.tile([128, 512], f32, tag="acc", bufs=2, name="acc")
                        acc_tiles[(qb, p)] = acc_t[:, 0:8 * (DH + 1)].rearrange(
                            "p (a b) -> p a b", b=DH + 1
                        )
                    st = ps.tile([128, 2, QB], f32, tag="st", bufs=3, name="st")
                    for hh in range(2):
                        h = 2 * p + hh
                        nc.tensor.matmul(
                            out=st[:, hh, :],
                            lhsT=k_sb[32 * h:32 * (h + 1), JT * J:JT * (J + 1)],
                            rhs=q_sb[32 * h:32 * (h + 1), q0:q0 + QB],
                            start=True, stop=True,
                            tile_position=(32 * h, 0),
                        )
                    pt = ptp.tile([128, 2, QB], bf16, tag="pt", name="pt")
                    pt_tiles[i] = pt
                    dump_pt = _dbg and i == 0
                    exp_err += ACT_SHARE
                    if exp_err >= 1.0:
                        exp_err -= 1.0
                        nc.scalar.activation(
                            out=pt[:], in_=st[:],
                            func=mybir.ActivationFunctionType.Exp,
                            scale=SCALE,
                        )
                    else:
                        nc.vector.tensor_scalar(
                            out=pt[:].bitcast(i16), in0=st[:],
                            scalar1=EXP2_A, scalar2=EXP2_B,
                            op0=mybir.AluOpType.mult,
                            op1=mybir.AluOpType.add,
                        )
                    if dump_pt:
                        dpt = wkp.tile([128, 2 * QB], f32, tag="dbgpt", name="dbgpt")
                        nc.vector.tensor_copy(out=dpt[:], in_=pt.rearrange("p a b -> p (a b)"))
                        nc.sync.dma_start(out=dbg_pt[:], in_=dpt[:])

                # PV for step i - PV_LAG
                j = i - PV_LAG
                if 0 <= j < nsteps:
                    qb, p, J = steps[j]
                    acc = acc_tiles[(qb, p)]
                    pt = pt_tiles.pop(j)
                    for hh in range(2):
                        h = 2 * p + hh
                        for qs in range(4):
                            # start=True resets has_written for the WHOLE psum
                            # bank (values persist), so only the first matmul
                            # into the bank may set it; later groups' J=0
                            # writes land on cleared bits and start fresh.
                            mm = nc.tensor.matmul(
                                out=acc[:, 2 * qs + hh, :],
                                lhsT=pt[:, hh, 128 * qs:128 * (qs + 1)],
                                rhs=vT3[:, HEADS * J + h, :],
                                start=(J == 0 and hh == 0 and qs == 0),
                                stop=(J == NJT - 1),
                                skip_group_check=True,
                            )
                            if pv_prev is not None:
                                add_dep_helper(mm.ins, pv_prev.ins, sync=False,
                                               reason="pv order")
                            pv_prev = mm
                    if J == NJT - 1:
                        tail_pair[i + 1] = (qb, p)
                        if p == 1:
                            tail_qb[i + (2 if qb == NQB - 1 else 4)] = qb

                # per-pair normalization as soon as a pair's PVs are done:
                # DVE copies acc->SBUF + reciprocal; GPSIMD scales (SBUF only)
                if i in tail_pair:
                    qb, p = tail_pair.pop(i)
                    acc = acc_tiles.pop((qb, p))
                    acc_sb = wkp.tile([128, 8, DH + 1], f32, tag="accsb", name="acc_sb")
                    if K_FIXED == "act2":
                        nc.scalar.copy(out=acc_sb[:], in_=acc[:])
                    else:
                        nc.vector.tensor_copy(out=acc_sb[:], in_=acc[:])
                    if _dbg and qb == 0 and p == 0:
                        nc.sync.dma_start(out=dbg_acc[:], in_=acc_sb.rearrange("p a b -> p (a b)"))
                    rcp_sb = wkp.tile([128, 8], f32, tag="rcp", name="rcp")
                    nc.vector.reciprocal(out=rcp_sb[:], in_=acc_sb[:, :, DH])
                    if qb not in atb_tiles:
                        atb_tiles[qb] = wkp.tile([128, 16, DH], bf16, tag="atb", name="atb")
                    atb = atb_tiles[qb]
                    for qs in range(4):
                        for hh in range(2):
                            nc.gpsimd.tensor_scalar_mul(
                                out=atb[:, 4 * qs + 2 * p + hh, :],
                                in0=acc_sb[:, 2 * qs + hh, 0:DH],
                                scalar1=rcp_sb[:, 2 * qs + hh:2 * qs + hh + 1],
                            )

                # per-qb finish: transpose A^T -> A, project, bias, DMA out
                if i in tail_qb:
                    qb = tail_qb.pop(i)
                    q0 = QB * qb
                    atb = atb_tiles.pop(qb)
                    a_ps_t = ps.tile([128, 2, 512], f32, tag="st", bufs=3, name="a_ps")
                    a_ps = a_ps_t[:, 0, :].bitcast(bf16).rearrange("p (a b) -> p a b", b=128)
                    atb_flat = atb.rearrange("p i d -> p (i d)")
                    for qs in range(4):
                        nc.tensor.transpose(
                            out=a_ps[:, qs:qs + 1, :],
                            in_=atb_flat[:, 128 * qs:128 * (qs + 1)],
                            identity=eye_sb[:],
                        )
                    a_sb = wkp.tile([128, 4, 128], bf16, tag="asb", name="a_sb")
                    if K_FIXED == "act2":
                        nc.scalar.copy(out=a_sb[:], in_=a_ps[:, 0:4, :])
                    else:
                        nc.vector.tensor_copy(out=a_sb[:], in_=a_ps[:, 0:4, :])
                    if _dbg and qb == 0:
                        datb = wkp.tile([128, 16 * DH], f32, tag="dbgatb", name="dbgatb")
                        nc.vector.tensor_copy(out=datb[:], in_=atb.rearrange("p a b -> p (a b)"))
                        nc.sync.dma_start(out=dbg_atb[:], in_=datb[:])
                        da = wkp.tile([128, 512], f32, tag="dbga", name="dbga")
                        nc.vector.tensor_copy(out=da[:], in_=a_sb.rearrange("p a b -> p (a b)"))
                        nc.sync.dma_start(out=dbg_a[:], in_=da[:])
                    a_flat = a_sb.rearrange("p a b -> p (a b)")
                    for cb in range(2):
                        op = ps.tile([128, 2, QB], f32, tag="st", bufs=3, name="op")[:, 0, :]
                        nc.tensor.matmul(
                            out=op[:],
                            lhsT=wo_sb[:, 128 * cb:128 * (cb + 1)],
                            rhs=a_flat[:],
                            start=True, stop=True,
                        )
                        ob = wkp.tile([128, QB], f32, tag="ob", name="ob")
                        if (K_FIXED == "mix") and cb == 0:
                            nc.scalar.add(out=ob[:], in_=op[:], add=bias_sb[:, cb:cb + 1])
                        elif K_FIXED == "act2" and cb == 0:
                            nc.scalar.add(out=ob[:], in_=op[:], add=bias_sb[:, cb:cb + 1])
                        else:
                            nc.vector.tensor_scalar_add(out=ob[:], in0=op[:], scalar1=bias_sb[:, cb:cb + 1])
                        nc.sync.dma_start(
                            out=out_d[128 * cb:128 * (cb + 1), q0:q0 + QB], in_=ob[:]
                        )

            # flush any remaining tails
            for i in sorted(list(tail_pair) + list(tail_qb)):
                assert False, "tails must be drained inside the loop"

            if _dbg:
                dk = wkp.tile([128, N], f32, tag="dbgk", name="dbgk")
                nc.vector.tensor_copy(out=dk[:], in_=k_sb[:])
                nc.sync.dma_start(out=dbg_k[:], in_=dk[:])
                dq = wkp.tile([128, NQ], f32, tag="dbgq", name="dbgq")
                nc.vector.tensor_copy(out=dq[:], in_=q_sb[:])
                nc.sync.dma_start(out=dbg_q[:], in_=dq[:])
                dv3 = wkp.tile([128, NJT * HEADS * (DH + 1)], f32, tag="dbgv", name="dbgv")
                nc.vector.tensor_copy(out=dv3[:], in_=vT3.rearrange("p a b -> p (a b)"))
                nc.sync.dma_start(out=dbg_v3[:], in_=dv3[:])



    nc.compile()
    return nc


_NC_CACHE = []


def _get_nc():
    if not _NC_CACHE:
        _NC_CACHE.append(build_nc())
    return _NC_CACHE[0]


def _make_in_maps(x, Wq, Wk, Wv, Wout, bout):
    import ml_dtypes

    bfl = ml_dtypes.bfloat16
    xf = np.ascontiguousarray(x.reshape(B, C, N)).astype(bfl)
    wqkv = np.ascontiguousarray(np.concatenate(
        [np.asarray(w, dtype=np.float32).T for w in (Wq, Wk, Wv)], axis=1
    ).astype(bfl))
    eyewo = np.ascontiguousarray(np.concatenate(
        [np.eye(128, dtype=np.float32),
         np.asarray(Wout, dtype=np.float32).T], axis=1
    ).astype(bfl))
    biasT = np.ascontiguousarray(
        np.asarray(bout, dtype=np.float32).reshape(2, 128).T
    )
    in_maps = []
    for core in range(8):
        b, half = core // 2, core % 2
        q0 = half * NQ
        in_maps.append({
            "x": xf[b],
            "xq": np.ascontiguousarray(xf[b][:, q0:q0 + NQ]),
            "wqkv": wqkv, "eyewo": eyewo, "biasT": biasT,
        })
    return in_maps


def kernel(x, Wq, Wk, Wv, Wout, bout):
    nc = _get_nc()
    in_maps = _make_in_maps(x, Wq, Wk, Wv, Wout, bout)
    res = run_bass_kernel_spmd(nc, in_maps, core_ids=list(range(8)))
    out = np.empty((B, C, N), dtype=np.float32)
    for core in range(8):
        b, half = core // 2, core % 2
        q0 = half * NQ
        out[b][:, q0:q0 + NQ] = res.results[core]["out"]
    return out.reshape(B, C, 64, 64)


# revision 29
# speedup vs baseline: 4.8221x; 1.0060x over previous
"""Trainium2 Bass kernel for 4-head spatial self-attention.

Computation (per batch b):
    xf = x[b] reshaped [C=256, n=4096]
    q/k/v = Wq/Wk/Wv @ xf            -> [128, n]   (rows = 4 heads x 32 dims)
    S_h   = (q_h^T k_h) * 32^-0.5    -> [n, n] per head
    P     = exp(S)  (softmax without max-subtraction: logits are O(10), safe)
    A_h   = P_h V_h^T / rowsum       -> [n, 32]
    out   = Wout @ A + bout          -> [C, n]

Sharding: 8 cores = 4 batches x 2 query-halves. Each core handles all 4 heads
for one batch and 2048 queries vs all 4096 keys; outputs are disjoint slices.

Perf notes (cost-model driven):
 - All matmul operands are float32r or bf16: 1 PE cycle per output free-row
   (fp32 costs 4).  float32r needs moving-free >= 256, so the small-free
   matmuls (PV, transposes) use bf16; q/k stay f32r for exact logits.
 - S^T is computed with keys on partitions (queries free) so exp(S^T) tiles
   feed PV directly as stationary operands.  The d=32 head contractions pack
   onto PE row strips via tile_position; concurrent strip matmuls must write
   different PSUM banks ([128, 2, 512] st tile, one bank per head).
 - PV is computed TRANSPOSED: A^T[128q, 33] += pt_slice.T @ [V^T | 1].  The
   moving operand is 33 wide (vs 512 the other way round).  Column 32 of the
   rhs is ones, so A^T col 32 accumulates the softmax denominator for free.
 - exp is split across ScalarE (true exp, PSUM->SBUF bf16) and DVE
   (Schraudolph exp2: one tensor_scalar mult+add writing int16 bits that
   reinterpret as bf16 ~= 2^y).
 - The (pair, J) stream is software-pipelined: PV lags S/exp by PV_LAG steps
   so the in-order PE queue never waits on the exp engines.
 - A^T normalization: DVE copies acc->SBUF + reciprocal of the denominator
   column; the 16 per-(head,qs) scale-multiplies run on otherwise-idle
   GPSIMD (SBUF only).  A^T is transposed back to [hd, q] with full-128
   PE transposes (bf16) and projected with Wout^T stationary.
"""

import numpy as np
import sys

for _p in ("/opt/trn_rl_repo", "/opt/pypackages"):
    if _p not in sys.path:
        sys.path.append(_p)

import concourse.bass as bass
import concourse.tile as tile
from concourse import bacc, mybir
from concourse.tile import add_dep_helper
from concourse.bass_utils import run_bass_kernel_spmd

f32 = mybir.dt.float32
f32r = mybir.dt.float32r
bf16 = mybir.dt.bfloat16
i16 = mybir.dt.int16

B = 4
C = 256
N = 4096          # h*w = 64*64 key positions
NQ = 2048         # queries per core (half batch)
HEADS = 4
DH = 32
INNER = 128
SCALE = DH ** -0.5

QB = 512          # query block (free dim of S^T tiles)
NQB = NQ // QB    # 4
JT = 128          # key tile (partition dim of S^T tiles)
NJT = N // JT     # 32

# Schraudolph exp2 constants for the bf16 bit pattern:
#   bf16_bits(e^(S*SCALE)) ~= round(128*(S*SCALE*log2(e)) + 128*(127-sigma))
EXP2_SIGMA = 0.0435
EXP2_A = 128.0 * SCALE * 1.4426950408889634
EXP2_B = 128.0 * (127.0 - EXP2_SIGMA)

ACT_SHARE = 0.50  # fraction of exp tiles on ScalarE (rest: DVE Schraudolph)
PV_LAG = 10       # steps PV trails S/exp in the software pipeline
K_VT3_ACT = False  # vT3 copies on DVE
K_HEAD = "serial"  # proj phase before the attention stream (engines idle there)
K_FIXED = "act2"   # k/q/v/acc/a copies on ScalarE; balances the DVE exp load


def build_nc():
    nc = bacc.Bacc()

    # x is host-rotated per core so this core's query half is cols [0, NQ);
    # key order is irrelevant (softmax sums over all keys).
    x_d = nc.dram_tensor("x", [C, N], bf16, kind="ExternalInput")
    wqkv_d = nc.dram_tensor("wqkv", [C, 3 * INNER], bf16, kind="ExternalInput")
    eyewo_d = nc.dram_tensor("eyewo", [128, 128 + C], bf16, kind="ExternalInput")
    biasT_d = nc.dram_tensor("biasT", [128, 2], f32, kind="ExternalInput")
    out_d = nc.dram_tensor("out", [C, NQ], f32, kind="ExternalOutput")
    import os
    _dbg = os.environ.get("KDBG", "0") == "1"
    if _dbg:
        dbg_k = nc.dram_tensor("dbg_k", [128, N], f32, kind="ExternalOutput")
        dbg_q = nc.dram_tensor("dbg_q", [128, NQ], f32, kind="ExternalOutput")
        dbg_v3 = nc.dram_tensor("dbg_v3", [128, NJT * HEADS * (DH + 1)], f32, kind="ExternalOutput")
        dbg_pt = nc.dram_tensor("dbg_pt", [128, 2 * QB], f32, kind="ExternalOutput")
        dbg_atb = nc.dram_tensor("dbg_atb", [128, 16 * DH], f32, kind="ExternalOutput")
        dbg_acc = nc.dram_tensor("dbg_acc", [128, 8 * (DH + 1)], f32, kind="ExternalOutput")
        dbg_a = nc.dram_tensor("dbg_a", [128, 512], f32, kind="ExternalOutput")

    with tile.TileContext(nc) as tc:
        import contextlib

        ctx = contextlib.ExitStack()
        with ctx:
            big = ctx.enter_context(tc.tile_pool(name="big", bufs=1))
            wkp = ctx.enter_context(tc.tile_pool(name="wkp", bufs=2))
            ptp = ctx.enter_context(tc.tile_pool(name="ptp", bufs=PV_LAG + 2))
            ps = ctx.enter_context(tc.tile_pool(name="ps", bufs=2, space="PSUM"))

            # ---- constants / weights (packed to minimize DMA count) ----
            wqkv_sb = big.tile([128, 2, 3 * INNER], bf16)  # [c_part, cc, (q|k|v)]
            eyewo_sb = big.tile([128, 128 + C], bf16)      # [inner, (eye|woT)]
            bias_sb = big.tile([128, 2], f32)
            wq_sb = wqkv_sb[:, :, 0:INNER]
            wk_sb = wqkv_sb[:, :, INNER:2 * INNER]
            wv_sb = wqkv_sb[:, :, 2 * INNER:3 * INNER]
            eye_sb = eyewo_sb[:, 0:128]
            wo_sb = eyewo_sb[:, 128:128 + C]

            # ---- activation DMA (chunked, interleaved with projections) ----
            x_sb = big.tile([128, 2, N], bf16)    # [c_part, c_chunk, n]

            k_sb = big.tile([128, N], f32r)       # [inner, n]
            q_sb = big.tile([128, NQ], f32r)      # [inner, nq]
            v_sb = big.tile([128, N], bf16)       # [inner, n]
            # v^T + ones col: [j0, (jtile, head), 33]; col 32 stays 1.0
            vT3 = big.tile([128, NJT * HEADS, DH + 1], bf16)
            nc.vector.memset(vT3[:, :, DH:DH + 1], 1.0)

            # ---- DMA priority order: weights -> x chunk 0 -> xq -> x chunk 1
            # -> eye/wo/bias.  k projection starts as soon as chunk 0 lands;
            # q follows xq; v waits for eye anyway. ----
            def emit_x_dma(ch):
                c0 = 1024 * ch
                for cc in range(2):
                    nc.sync.dma_start(
                        out=x_sb[:, cc, c0:c0 + 1024],
                        in_=x_d[128 * cc:128 * (cc + 1), c0:c0 + 1024],
                    )

            for cc in range(2):
                nc.sync.dma_start(out=wqkv_sb[:, cc, :], in_=wqkv_d[128 * cc:128 * (cc + 1), :])
            emit_x_dma(0)
            nc.sync.dma_start(out=eyewo_sb[:], in_=eyewo_d[:])
            nc.sync.dma_start(out=bias_sb[:], in_=biasT_d[:])
            for ch in range(1, 4):
                emit_x_dma(ch)

            def emit_q_tile(t):
                qp = ps.tile([128, 2, 512], f32, tag="st", bufs=3, name="qp")[:, 0, :]
                for cc in range(2):
                    nc.tensor.matmul(
                        out=qp[:],
                        lhsT=wq_sb[:, cc, :],
                        rhs=x_sb[:, cc, 512 * t:512 * (t + 1)],
                        start=(cc == 0), stop=(cc == 1),
                    )
                if K_FIXED in ("mix", "act2"):
                    nc.scalar.copy(out=q_sb[:, 512 * t:512 * (t + 1)], in_=qp[:])
                else:
                    nc.vector.tensor_copy(out=q_sb[:, 512 * t:512 * (t + 1)], in_=qp[:])

            for t in range(NQ // 512):
                emit_q_tile(t)

            def emit_k_tile(t):
                kp = ps.tile([128, 2, 512], f32, tag="st", bufs=3, name="kp")[:, 0, :]
                for cc in range(2):
                    nc.tensor.matmul(
                        out=kp[:],
                        lhsT=wk_sb[:, cc, :],
                        rhs=x_sb[:, cc, 512 * t:512 * (t + 1)],
                        start=(cc == 0), stop=(cc == 1),
                    )
                if K_FIXED == "act2" or (K_FIXED == "mix" and t % 2 == 0):
                    nc.scalar.copy(out=k_sb[:, 512 * t:512 * (t + 1)], in_=kp[:])
                else:
                    nc.vector.tensor_copy(out=k_sb[:, 512 * t:512 * (t + 1)], in_=kp[:])

            def emit_v_tile(t):
                # v projection for n-cols [512t, 512t+512), then transpose into
                # vT3 (full-128 transposes, no row-strip hazards)
                vp = ps.tile([128, 2, 512], f32, tag="st", bufs=3, name="vp")[:, 0, :]
                for cc in range(2):
                    nc.tensor.matmul(
                        out=vp[:],
                        lhsT=wv_sb[:, cc, :],
                        rhs=x_sb[:, cc, 512 * t:512 * (t + 1)],
                        start=(cc == 0), stop=(cc == 1),
                    )
                if K_FIXED == "act2":
                    nc.scalar.copy(out=v_sb[:, 512 * t:512 * (t + 1)], in_=vp[:])
                else:
                    nc.vector.tensor_copy(out=v_sb[:, 512 * t:512 * (t + 1)], in_=vp[:])
                vp2_t = ps.tile([128, 2, 512], f32, tag="st", bufs=3, name="vp2")
                vp2 = vp2_t[:, 0, :].bitcast(bf16).rearrange("p (a b) -> p a b", b=128)
                for j2 in range(4):
                    j = 4 * t + j2
                    nc.tensor.transpose(
                        out=vp2[:, j2:j2 + 1, :],
                        in_=v_sb[:, 128 * j:128 * (j + 1)],
                        identity=eye_sb[:],
                    )
                src = vp2[:, 0:4, :].rearrange("p j (h d) -> p (j h) d", d=DH)
                eng = nc.scalar.copy if K_VT3_ACT else nc.vector.tensor_copy
                eng(out=vT3[:, 16 * t:16 * (t + 1), 0:DH], in_=src)

            if K_HEAD == "serial":
                for t in range(8):
                    emit_k_tile(t)
                    emit_v_tile(t)

            # ---- attention: software-pipelined (qb, p, J) stream ----
            steps = [(qb, p, J) for qb in range(NQB) for p in range(2)
                     for J in range(NJT)]
            nsteps = len(steps)
            exp_err = 0.0
            pt_tiles = {}     # step idx -> pt tile
            acc_tiles = {}    # (qb, p) -> acc psum tile
            atb_tiles = {}    # qb -> normalized A^T sbuf tile
            pv_prev = None
            # scheduled tail work: emitted after the S/PV of the given step
            tail_pair = {}    # step idx -> (qb, p) whose PVs just finished
            tail_qb = {}      # step idx -> qb ready for transpose/proj/out

            for i in range(nsteps + PV_LAG + 5):
                # lazy k/v projection work: k tile t at step 2t, v tile t at
                # step 2t+1, next x chunk ahead of need

                # S^T + exp for step i
                if i < nsteps:
                    qb, p, J = steps[i]
                    q0 = QB * qb
                    if (qb, p) not in acc_tiles:
                        acc_t = ps.tile([128, 512], f32, tag="acc", bufs=2, name="acc")
                        acc_tiles[(qb, p)] = acc_t[:, 0:8 * (DH + 1)].rearrange(
                            "p (a b) -> p a b", b=DH + 1
                        )
                    st = ps.tile([128, 2, QB], f32, tag="st", bufs=3, name="st")
                    for hh in range(2):
                        h = 2 * p + hh
                        nc.tensor.matmul(
                            out=st[:, hh, :],
                            lhsT=k_sb[32 * h:32 * (h + 1), JT * J:JT * (J + 1)],
                            rhs=q_sb[32 * h:32 * (h + 1), q0:q0 + QB],
                            start=True, stop=True,
                            tile_position=(32 * h, 0),
                        )
                    pt = ptp.tile([128, 2, QB], bf16, tag="pt", name="pt")
                    pt_tiles[i] = pt
                    dump_pt = _dbg and i == 0
                    exp_err += ACT_SHARE
                    if exp_err >= 1.0:
                        exp_err -= 1.0
                        nc.scalar.activation(
                            out=pt[:], in_=st[:],
                            func=mybir.ActivationFunctionType.Exp,
                            scale=SCALE,
                        )
                    else:
                        nc.vector.tensor_scalar(
                            out=pt[:].bitcast(i16), in0=st[:],
                            scalar1=EXP2_A, scalar2=EXP2_B,
                            op0=mybir.AluOpType.mult,
                            op1=mybir.AluOpType.add,
                        )
                    if dump_pt:
                        dpt = wkp.tile([128, 2 * QB], f32, tag="dbgpt", name="dbgpt")
                        nc.vector.tensor_copy(out=dpt[:], in_=pt.rearrange("p a b -> p (a b)"))
                        nc.sync.dma_start(out=dbg_pt[:], in_=dpt[:])

                # PV for step i - PV_LAG
                j = i - PV_LAG
                if 0 <= j < nsteps:
                    qb, p, J = steps[j]
                    acc = acc_tiles[(qb, p)]
                    pt = pt_tiles.pop(j)
                    for hh in range(2):
                        h = 2 * p + hh
                        for qs in range(4):
                            # start=True resets has_written for the WHOLE psum
                            # bank (values persist), so only the first matmul
                            # into the bank may set it; later groups' J=0
                            # writes land on cleared bits and start fresh.
                            mm = nc.tensor.matmul(
                                out=acc[:, 2 * qs + hh, :],
                                lhsT=pt[:, hh, 128 * qs:128 * (qs + 1)],
                                rhs=vT3[:, HEADS * J + h, :],
                                start=(J == 0 and hh == 0 and qs == 0),
                                stop=(J == NJT - 1),
                                skip_group_check=True,
                            )
                            if pv_prev is not None:
                                add_dep_helper(mm.ins, pv_prev.ins, sync=False,
                                               reason="pv order")
                            pv_prev = mm
                    if J == NJT - 1:
                        tail_pair[i + 1] = (qb, p)
                        if p == 1:
                            tail_qb[i + (2 if qb == NQB - 1 else 4)] = qb

                # per-pair normalization as soon as a pair's PVs are done:
                # DVE copies acc->SBUF + reciprocal; GPSIMD scales (SBUF only)
                if i in tail_pair:
                    qb, p = tail_pair.pop(i)
                    acc = acc_tiles.pop((qb, p))
                    acc_sb = wkp.tile([128, 8, DH + 1], f32, tag="accsb", name="acc_sb")
                    if K_FIXED == "act2":
                        nc.scalar.copy(out=acc_sb[:], in_=acc[:])
                    else:
                        nc.vector.tensor_copy(out=acc_sb[:], in_=acc[:])
                    if _dbg and qb == 0 and p == 0:
                        nc.sync.dma_start(out=dbg_acc[:], in_=acc_sb.rearrange("p a b -> p (a b)"))
                    rcp_sb = wkp.tile([128, 8], f32, tag="rcp", name="rcp")
                    nc.vector.reciprocal(out=rcp_sb[:], in_=acc_sb[:, :, DH])
                    if qb not in atb_tiles:
                        atb_tiles[qb] = wkp.tile([128, 16, DH], bf16, tag="atb", name="atb")
                    atb = atb_tiles[qb]
                    for qs in range(4):
                        for hh in range(2):
                            nc.gpsimd.tensor_scalar_mul(
                                out=atb[:, 4 * qs + 2 * p + hh, :],
                                in0=acc_sb[:, 2 * qs + hh, 0:DH],
                                scalar1=rcp_sb[:, 2 * qs + hh:2 * qs + hh + 1],
                            )

                # per-qb finish: transpose A^T -> A, project, bias, DMA out
                if i in tail_qb:
                    qb = tail_qb.pop(i)
                    q0 = QB * qb
                    atb = atb_tiles.pop(qb)
                    a_ps_t = ps.tile([128, 2, 512], f32, tag="st", bufs=3, name="a_ps")
                    a_ps = a_ps_t[:, 0, :].bitcast(bf16).rearrange("p (a b) -> p a b", b=128)
                    atb_flat = atb.rearrange("p i d -> p (i d)")
                    for qs in range(4):
                        nc.tensor.transpose(
                            out=a_ps[:, qs:qs + 1, :],
                            in_=atb_flat[:, 128 * qs:128 * (qs + 1)],
                            identity=eye_sb[:],
                        )
                    a_sb = wkp.tile([128, 4, 128], bf16, tag="asb", name="a_sb")
                    if K_FIXED == "act2":
                        nc.scalar.copy(out=a_sb[:], in_=a_ps[:, 0:4, :])
                    else:
                        nc.vector.tensor_copy(out=a_sb[:], in_=a_ps[:, 0:4, :])
                    if _dbg and qb == 0:
                        datb = wkp.tile([128, 16 * DH], f32, tag="dbgatb", name="dbgatb")
                        nc.vector.tensor_copy(out=datb[:], in_=atb.rearrange("p a b -> p (a b)"))
                        nc.sync.dma_start(out=dbg_atb[:], in_=datb[:])
                        da = wkp.tile([128, 512], f32, tag="dbga", name="dbga")
                        nc.vector.tensor_copy(out=da[:], in_=a_sb.rearrange("p a b -> p (a b)"))
                        nc.sync.dma_start(out=dbg_a[:], in_=da[:])
                    a_flat = a_sb.rearrange("p a b -> p (a b)")
                    for cb in range(2):
                        op = ps.tile([128, 2, QB], f32, tag="st", bufs=3, name="op")[:, 0, :]
                        nc.tensor.matmul(
                            out=op[:],
                            lhsT=wo_sb[:, 128 * cb:128 * (cb + 1)],
                            rhs=a_flat[:],
                            start=True, stop=True,
                        )
                        ob = wkp.tile([128, QB], f32, tag="ob", name="ob")
                        if (K_FIXED == "mix") and cb == 0:
                            nc.scalar.add(out=ob[:], in_=op[:], add=bias_sb[:, cb:cb + 1])
                        elif K_FIXED == "act2" and cb == 0:
                            nc.scalar.add(out=ob[:], in_=op[:], add=bias_sb[:, cb:cb + 1])
                        else:
                            nc.vector.tensor_scalar_add(out=ob[:], in0=op[:], scalar1=bias_sb[:, cb:cb + 1])
                        nc.sync.dma_start(
                            out=out_d[128 * cb:128 * (cb + 1), q0:q0 + QB], in_=ob[:]
                        )

            # flush any remaining tails
            for i in sorted(list(tail_pair) + list(tail_qb)):
                assert False, "tails must be drained inside the loop"

            if _dbg:
                dk = wkp.tile([128, N], f32, tag="dbgk", name="dbgk")
                nc.vector.tensor_copy(out=dk[:], in_=k_sb[:])
                nc.sync.dma_start(out=dbg_k[:], in_=dk[:])
                dq = wkp.tile([128, NQ], f32, tag="dbgq", name="dbgq")
                nc.vector.tensor_copy(out=dq[:], in_=q_sb[:])
                nc.sync.dma_start(out=dbg_q[:], in_=dq[:])
                dv3 = wkp.tile([128, NJT * HEADS * (DH + 1)], f32, tag="dbgv", name="dbgv")
                nc.vector.tensor_copy(out=dv3[:], in_=vT3.rearrange("p a b -> p (a b)"))
                nc.sync.dma_start(out=dbg_v3[:], in_=dv3[:])



    nc.compile()
    return nc


_NC_CACHE = []


def _get_nc():
    if not _NC_CACHE:
        _NC_CACHE.append(build_nc())
    return _NC_CACHE[0]


def _make_in_maps(x, Wq, Wk, Wv, Wout, bout):
    import ml_dtypes

    bfl = ml_dtypes.bfloat16
    xf = np.ascontiguousarray(x.reshape(B, C, N)).astype(bfl)
    wqkv = np.ascontiguousarray(np.concatenate(
        [np.asarray(w, dtype=np.float32).T for w in (Wq, Wk, Wv)], axis=1
    ).astype(bfl))
    eyewo = np.ascontiguousarray(np.concatenate(
        [np.eye(128, dtype=np.float32),
         np.asarray(Wout, dtype=np.float32).T], axis=1
    ).astype(bfl))
    biasT = np.ascontiguousarray(
        np.asarray(bout, dtype=np.float32).reshape(2, 128).T
    )
    in_maps = []
    for core in range(8):
        b, half = core // 2, core % 2
        q0 = half * NQ
        xr = xf[b] if q0 == 0 else np.ascontiguousarray(np.roll(xf[b], -q0, axis=1))
        in_maps.append({
            "x": xr,
            "wqkv": wqkv, "eyewo": eyewo, "biasT": biasT,
        })
    return in_maps


def kernel(x, Wq, Wk, Wv, Wout, bout):
    nc = _get_nc()
    in_maps = _make_in_maps(x, Wq, Wk, Wv, Wout, bout)
    res = run_bass_kernel_spmd(nc, in_maps, core_ids=list(range(8)))
    out = np.empty((B, C, N), dtype=np.float32)
    for core in range(8):
        b, half = core // 2, core % 2
        q0 = half * NQ
        out[b][:, q0:q0 + NQ] = res.results[core]["out"]
    return out.reshape(B, C, 64, 64)
